# revision 30
# baseline (speedup 1.0000x reference)
"""Trainium2 Bass kernel for nn_Attention_90074054132266.

Full multi-head attention (B=2, S=4096, D=512, H=8, HD=64) with RoPE on
q/k, sharded over 8 NeuronCores: batch x head-pair (data parallel over
batch, tensor parallel over heads; core c handles batch c//4, heads
2*(c%4), 2*(c%4)+1). Each core computes a partial output projection
(its 2 heads' contribution); the host sums the 4 per-batch partials
(the "all-reduce") and adds wo_b.

Per-core device algorithm (everything stored transposed, f32/bf16):
  - host passes x[b].T, so projections q^T/k^T = wq^T-chunks @ x^T run
    as N=512 matmuls.
  - RoPE via duplicated projections with half-swapped weight columns
    (q2^T[d] = q^T[(d+32)%64 per head]) + sign-baked cos/sin tables:
    q_rot = q^T * cosf + q2^T * sinf  (3 VectorE tensor-tensor ops).
  - scores computed transposed per 128-k-chunk for BOTH heads into one
    PSUM tile S^T[k, (h, q)] = [128, 1024]; exp(S/8) on ScalarE out of
    PSUM (scale folded into the activation; scores ~ N(0,1), exp safe).
  - P@V accumulates per-head O'^T[65, 512] over the 32 k-chunks; V' has
    a ones column appended per head: row 64 = softmax denominator Z.
  - Z path: copy Z rows, reciprocal, bounce via DRAM into a partition-
    broadcast tile izb[128, 512] (rows 0:64 = 1/Z_h0, 64:128 = 1/Z_h1);
    O' evacuated as O'*(1/Z) into a PACKED ot[128, S] (h0 rows 0:64,
    h1 rows 64:128).  No Z transpose, no post-projection scaling.
  - output projection is then a SINGLE matmul per 128-row tile:
    U = ot[:, us].T @ wo (contraction covers both heads), evac + DMA.
  - k/v projection is interleaved with qt=0's attention (chunk sc feeds
    k-chunks 4sc..4sc+3) so ScalarE starts exp'ing ~5us into the kernel
    instead of after the full projection phase.
"""

import os
import sys

sys.path.insert(0, "/opt/trn_rl_repo")

import numpy as np

B, S, DIM, HEADS, HD = 2, 4096, 512, 8, 64
HALF = HD // 2
NCORES = 8
HPC = 2  # heads per core
DPC = HPC * HD  # 128 projection columns per core
NSC = S // 512  # 8 q-column chunks of 512
NKC = S // 128  # 32 k-chunks of 128
NUT = S // 128  # 32 q-row tiles of 128

_CACHE = {}


def _split_multiwait_drains(nc):
    """The walrus build in this container rejects any instruction with
    more than one sync-wait ("Too many sync wait commands"). Hoist the
    extra waits onto preceding same-engine NoOps, leaving one wait on
    the original instruction."""
    import bass_rust
    import concourse.mybir as mybir

    for fn in nc.m.functions:
        for bb in fn.blocks:
            new_insts = []
            changed = False
            for inst in bb.instructions:
                si = getattr(inst, "sync_info", None)
                if si is not None and len(si.on_wait) > 1:
                    waits = list(si.on_wait)
                    for k, w in enumerate(waits[:-1]):
                        d = mybir.InstNoOp(name=f"{inst.name}w{k}", ins=[], outs=[])
                        d.engine = inst.engine
                        d.sync_info = bass_rust.SyncInfo(on_wait=[w], on_update=[])
                        new_insts.append(d)
                    inst.sync_info = bass_rust.SyncInfo(
                        on_wait=[waits[-1]], on_update=list(si.on_update)
                    )
                    changed = True
                new_insts.append(inst)
            if changed:
                bb.instructions = new_insts


def _build(qk_bias, v_bias, use_bf16=True):
    import concourse.bass as bass
    import concourse.tile as tile
    from concourse import mybir

    F32 = mybir.dt.float32
    F32R = mybir.dt.float32r
    BF16 = mybir.dt.bfloat16
    MMD = BF16 if use_bf16 else F32R   # matmul operand dtype (SBUF tiles)
    MME = BF16 if use_bf16 else F32    # DRAM dtype for matmul inputs
    EXP = mybir.ActivationFunctionType.Exp
    MUL = mybir.AluOpType.mult
    ADD = mybir.AluOpType.add
    I16 = mybir.dt.int16
    # Schraudolph fast-exp in bf16: bitcast(int16(A*x + B)) ~= exp(x/8)
    # (max rel err ~3%; used on a few tiles per q-chunk to offload the
    # saturated ScalarE onto the idle VectorE)
    SCH_A = float(2.0**7 / np.log(2.0) * 0.125)
    SCH_B = float(127.0 * 2.0**7 - 366393.0 / 65536.0)

    nc = bass.Bass("TRN2")

    xt_e = nc.declare_dram_parameter("xt", [DIM, S], MME, isOutput=False)
    w_e = {}
    for name in ("wq", "wqp", "wk", "wkp", "wv"):
        w_e[name] = nc.declare_dram_parameter(name, [DIM, DPC], MME, isOutput=False)
    wo_e = nc.declare_dram_parameter("wo", [DPC, DIM], MME, isOutput=False)
    cos_e = nc.declare_dram_parameter("cosf", [DPC, S], F32, isOutput=False)
    sin_e = nc.declare_dram_parameter("sinf", [DPC, S], F32, isOutput=False)
    b_e = {}
    if qk_bias:
        for name in ("qb", "qbp", "kb", "kbp"):
            b_e[name] = nc.declare_dram_parameter(name, [DPC, 1], F32, isOutput=False)
    if v_bias:
        b_e["vb"] = nc.declare_dram_parameter("vb", [1, DPC], F32, isOutput=False)
    out_e = nc.declare_dram_parameter("out", [S, DIM], F32, isOutput=True)

    with tile.TileContext(nc) as tc:
        with (
            tc.tile_pool(name="persist", bufs=1) as P,
            tc.tile_pool(name="work", bufs=2) as W,
            tc.tile_pool(name="ptp", bufs=8) as PT,
        ):
            # ---- persistent SBUF tensors ----
            qr = P.tile([DPC, S], MMD, tag="qr")  # rotated q^T
            # rotated k^T, zero-padded per head to full K=128 contraction
            # (row-masked K=64 matmuls don't count as PE-busy for the HAM
            # clock gate; mixing them with PV pins the PE at 1.2 GHz)
            krA = P.tile([DPC, S], MMD, tag="krA")
            krB = P.tile([DPC, S], MMD, tag="krB")
            # Z rows staging (head h at partition 32h; DVE outputs must be
            # quadrant-aligned)
            zrow = P.tile([33, S], F32, tag="zrow")
            # V' rows: per k-chunk st, V[k, :] for head A cols 0:64 + ones
            # col 64, head B cols 65:129 + ones col 129.
            VW = 2 * (HD + 1)  # 130
            vb_sb = P.tile([128, NKC, VW], MMD, tag="vboth")
            # normalized O^T packed: rows 0:64 head A, 64:128 head B
            ot = P.tile([DPC, S], MMD, tag="ot")
            wo_sb = P.tile([DPC, DIM], MMD, tag="wo")


            bias_sb = {}
            if qk_bias:
                for name in ("qb", "qbp", "kb", "kbp"):
                    t = P.tile([DPC, 1], F32, tag=name)
                    nc.sync.dma_start(out=t, in_=b_e[name][:])
                    bias_sb[name] = t
            if v_bias:
                vbias_bc = P.tile([128, DPC], F32, tag="vbias")
                src = bass.AP(
                    tensor=b_e["vb"].tensor,
                    offset=b_e["vb"].offset,
                    ap=[[0, 128], [1, DPC]],
                )
                nc.sync.dma_start(out=vbias_bc, in_=src)

            # ---- PSUM budget (8 banks): scores 2x2 + pso 2x1 + proj 1
            # + out-proj U 1 (separate pools so filler rotations decouple)
            with (
                tc.tile_pool(name="xtp", bufs=2) as XT,
                tc.tile_pool(name="wpool", bufs=1) as WP,
                tc.tile_pool(name="psp", bufs=1, space="PSUM") as PSP,
                tc.tile_pool(name="psu", bufs=1, space="PSUM") as PSU,
                tc.tile_pool(name="pss", bufs=2, space="PSUM") as PSS,
                tc.tile_pool(name="pso", bufs=1, space="PSUM") as PSO,
            ):
                cos_sb = WP.tile([DPC, S], F32, tag="cos")
                sin_sb = WP.tile([DPC, S], F32, tag="sin")

                def load_tables(sc):
                    qs_ = bass.ts(sc, 512)
                    nc.sync.dma_start(out=cos_sb[:, qs_], in_=cos_e[:, qs_])
                    nc.sync.dma_start(out=sin_sb[:, qs_], in_=sin_e[:, qs_])

                w_sb = {}

                def load_w(name):
                    t = WP.tile([128, 4, DPC], MMD, tag=name)
                    nc.sync.dma_start(
                        out=t,
                        in_=(
                            w_e[name][:].rearrange("(c p) m -> p c m", p=128)
                            if use_bf16
                            else w_e[name][:]
                            .rearrange("(c p) m -> p c m", p=128)
                            .bitcast(F32R)
                        ),
                    )
                    w_sb[name] = t

                xt_r = xt_e[:].rearrange("(c p) s -> c p s", p=128)

                def load_xt(sc, qs):
                    xt_c = []
                    for c in range(4):
                        t = XT.tile(
                            [128, 512], MMD, tag=f"xt{c}", name=f"xt{c}_{sc}"
                        )
                        nc.sync.dma_start(
                            out=t,
                            in_=xt_r[c, :, qs]
                            if use_bf16
                            else xt_r[c, :, qs].bitcast(F32R),
                        )
                        xt_c.append(t)
                    return xt_c

                def rope_proj(xt_c, qs, which, pools=None):
                    pools = pools or (PSP, PSP)
                    # one 512-col chunk of rotated q^T or (split) k^T
                    wn, wpn, bn, bpn = (
                        ("wq", "wqp", "qb", "qbp")
                        if which == "q"
                        else ("wk", "wkp", "kb", "kbp")
                    )
                    ps1 = pools[0].tile(
                        [128, 512], F32, tag=pools[0].name[-3:], name=f"p1_{which}{qs}"
                    )
                    for c in range(4):
                        nc.tensor.matmul(
                            ps1,
                            w_sb[wn][:, c, :],
                            xt_c[c][:],
                            start=(c == 0),
                            stop=(c == 3),
                        )
                    if qk_bias:
                        s1 = W.tile([128, 512], F32, tag="rope1")
                        nc.vector.tensor_scalar_add(s1, ps1, bias_sb[bn])
                    else:
                        s1 = ps1
                    t3 = W.tile([128, 512], F32, tag="rope3")
                    nc.vector.tensor_tensor(out=t3, in0=s1, in1=cos_sb[:, qs], op=MUL)
                    ps2 = pools[1].tile(
                        [128, 512], F32, tag=pools[1].name[-3:], name=f"p2_{which}{qs}"
                    )
                    for c in range(4):
                        nc.tensor.matmul(
                            ps2,
                            w_sb[wpn][:, c, :],
                            xt_c[c][:],
                            start=(c == 0),
                            stop=(c == 3),
                        )
                    if qk_bias:
                        s2 = W.tile([128, 512], F32, tag="rope2")
                        nc.vector.tensor_scalar_add(s2, ps2, bias_sb[bpn])
                    else:
                        s2 = ps2
                    t4 = W.tile([128, 512], F32, tag="rope4")
                    nc.vector.tensor_tensor(out=t4, in0=s2, in1=sin_sb[:, qs], op=MUL)
                    if which == "q":
                        nc.vector.tensor_tensor(out=qr[:, qs], in0=t3, in1=t4, op=ADD)
                    else:
                        nc.vector.tensor_tensor(
                            out=krA[0:HD, qs], in0=t3[0:HD, :], in1=t4[0:HD, :], op=ADD
                        )
                        nc.vector.tensor_tensor(
                            out=krB[HD:DPC, qs],
                            in0=t3[HD:DPC, :],
                            in1=t4[HD:DPC, :],
                            op=ADD,
                        )

                def v_proj_part(xt_c, sc, stl, pl):
                    st = sc * 4 + stl
                    psv = pl.tile([128, 512], F32, tag=pl.name[-3:], name=f"pv{st}")
                    for c in range(4):
                        nc.tensor.matmul(
                            psv[:, 0:128],
                            xt_c[c][:, bass.ts(stl, 128)],
                            w_sb["wv"][:, c, :],
                            start=(c == 0),
                            stop=(c == 3),
                        )
                    dsts = vb_sb[:, st, :].rearrange(
                        "p (j w) -> p j w", w=HD + 1
                    )[:, :, 0:HD]
                    if v_bias:
                        nc.vector.tensor_tensor(
                            out=dsts, in0=psv[:, 0:128], in1=vbias_bc, op=ADD
                        )
                    else:
                        nc.vector.tensor_copy(out=dsts, in_=psv[:, 0:128])

                # DRAM scratch for the 1/Z partition-broadcast bounce
                zscr = nc.dram_tensor("zscr", [HPC, S], F32)
                zscr_ap = zscr[:]

                def scores_exp(kc, qs, qt, dve_exp=False):
                    # scores for both heads of k-chunk kc, then exp
                    pss_t = PSS.tile([128, 1024], F32, tag="s", name=f"ss{qt}_{kc}")
                    nc.tensor.matmul(
                        pss_t[:, 0:512],
                        krA[:, bass.ts(kc, 128)],
                        qr[:, qs],
                        start=True,
                        stop=True,
                    )
                    nc.tensor.matmul(
                        pss_t[:, 512:1024],
                        krB[:, bass.ts(kc, 128)],
                        qr[:, qs],
                        start=True,
                        stop=True,
                    )
                    if dve_exp:
                        pti = PT.tile(
                            [128, 1024], I16, tag="pti", name=f"pti{qt}_{kc}"
                        )
                        eng = nc.gpsimd if dve_exp == "gps" else nc.vector
                        eng.tensor_scalar(
                            out=pti,
                            in0=pss_t,
                            scalar1=SCH_A,
                            scalar2=SCH_B,
                            op0=MUL,
                            op1=ADD,
                        )
                        return pti[:].bitcast(BF16)
                    pt = PT.tile([128, 1024], MMD, tag="pt", name=f"pt{qt}_{kc}")
                    nc.scalar.activation(out=pt, in_=pss_t, func=EXP, scale=0.125)
                    return pt

                def attn_pv(pso, kc, pt):
                    for h in range(HPC):
                        nc.tensor.matmul(
                            pso[h],
                            vb_sb[:, kc, bass.ds(h * (HD + 1), HD + 1)],
                            pt[:, bass.ts(h, 512)],
                            start=(kc == 0),
                            stop=(kc == NKC - 1),
                        )

                def attn_kc(pso, kc, qs, qt):
                    attn_pv(pso, kc, scores_exp(kc, qs, qt))

                def evac(pso, qs, qt):
                    # fast PSUM evacuation: Z rows first (they gate the
                    # normalize DMA chain), then unnormalized O' into
                    # packed ot; releases the pso banks quickly so the
                    # next q-chunk's PV never stalls.
                    for h in range(HPC):
                        nc.vector.tensor_copy(
                            out=zrow[32 * h : 32 * h + 1, qs],
                            in_=pso[h][HD : HD + 1, :],
                        )
                    for h in range(HPC):
                        nc.vector.tensor_copy(
                            out=ot[h * HD : (h + 1) * HD, qs], in_=pso[h][0:HD, :]
                        )

                def normalize_project(qs, qt, stagger=False):
                    # Z -> DRAM -> partition-broadcast -> 1/Z, normalize ot
                    # in place, then the fused single-matmul out-projection.
                    # Latency-tolerant: runs as filler during the next qt.
                    for h in range(HPC):
                        nc.sync.dma_start(
                            out=zscr[h : h + 1, qs],
                            in_=zrow[32 * h : 32 * h + 1, qs],
                        )
                    izbz = W.tile([128, 512], F32, tag="izbz", name=f"izbz{qt}")
                    for h in range(HPC):
                        zsl = zscr_ap[h : h + 1, qs]
                        src = bass.AP(
                            tensor=zsl.tensor,
                            offset=zsl.offset,
                            ap=[[0, HD], [1, 512]],
                        )
                        nc.sync.dma_start(out=izbz[h * HD : (h + 1) * HD, :], in_=src)
                    izb = W.tile([128, 512], F32, tag="izb", name=f"izb{qt}")
                    for utl in range(4):
                        stk = (
                            tc.high_priority(offset=-(15 + 36 * utl))
                            if stagger
                            else None
                        )
                        if stk is not None:
                            stk.__enter__()
                        ut = qt * 4 + utl
                        us = bass.ts(ut, 128)
                        cs = bass.ts(utl, 128)
                        nc.vector.reciprocal(out=izb[:, cs], in_=izbz[:, cs])
                        for h in range(HPC):
                            otb = ot[h * HD : (h + 1) * HD, qs].rearrange(
                                "p (u c) -> p u c", c=128
                            )[:, utl, :]
                            nc.vector.tensor_tensor(
                                out=otb,
                                in0=otb,
                                in1=izb[h * HD : (h + 1) * HD, cs],
                                op=MUL,
                            )
                        psu = PSU.tile(
                            [128, DIM], F32, tag=PSU.name[-3:], name=f"u{ut}"
                        )
                        nc.tensor.matmul(
                            psu, ot[:, us], wo_sb[:, :], start=True, stop=True
                        )
                        t_out = W.tile([128, DIM], F32, tag="uout")
                        nc.vector.tensor_copy(out=t_out, in_=psu)
                        nc.sync.dma_start(out=out_e[us, :], in_=t_out)
                        if stk is not None:
                            stk.__exit__(None, None, None)

                # ---- phase 1: q0 proj, then k/v proj interleaved with
                # qt=0's attention (chunk sc feeds k-chunks 4sc..4sc+3).
                # DMAs are emitted in dependency order so q0/kv0 data
                # arrives first; kv-proj PSUM temps alternate the two
                # spare banks (PSP/PSU) to pipeline matmul vs. evac. ----
                load_tables(0)
                load_w("wq")
                load_w("wqp")
                xt_c = load_xt(0, bass.ts(0, 512))
                load_w("wk")
                load_w("wkp")
                load_w("wv")
                rope_proj(xt_c, bass.ts(0, 512), "q", pools=(PSP, PSU))
                qs0 = bass.ts(0, 512)
                pso = [
                    PSO.tile([HD + 1, 512], F32, tag=f"o{h}", name=f"o0_{h}")
                    for h in range(HPC)
                ]
                for sc in range(NSC):
                    qs = bass.ts(sc, 512)
                    if sc >= 1:
                        load_tables(sc)
                    xt_c = load_xt(8 + sc, qs)
                    # zero-pad rows of this chunk only (a full-tensor
                    # memset would hold up the first scores for ~16us)
                    nc.vector.memset(krA[HD:DPC, qs], 0.0)
                    nc.vector.memset(krB[0:HD, qs], 0.0)
                    ones_sc = vb_sb[:, bass.ds(sc * 4, 4), :].rearrange(
                        "p s (j w) -> p s j w", w=HD + 1
                    )[:, :, :, HD : HD + 1]
                    nc.vector.memset(
                        ones_sc if use_bf16 else ones_sc.bitcast(F32), 1.0
                    )
                    rope_proj(xt_c, qs, "k", pools=(PSP, PSU))
                    pts = []
                    for kcl in range(4):
                        pts.append(scores_exp(sc * 4 + kcl, qs0, 0))
                        v_proj_part(xt_c, sc, kcl, (PSP, PSU)[kcl % 2])
                        if kcl >= 1:
                            attn_pv(pso, sc * 4 + kcl - 1, pts[kcl - 1])
                    if sc < NSC - 1:
                        attn_pv(pso, sc * 4 + 3, pts[3])
                    if sc == 6:
                        # project q1 so qt=1 can start right after qt=0
                        # (normal priority; a mid-loop high_priority insert
                        # inverts proj pool rotation order and deadlocks)
                        xt_q = load_xt(16, bass.ts(1, 512))
                        rope_proj(xt_q, bass.ts(1, 512), "q", pools=(PSP, PSU))
                nc.sync.dma_start(
                    out=wo_sb, in_=wo_e[:] if use_bf16 else wo_e[:].bitcast(F32R)
                )
                pend = (pso, {NKC - 1: pts[3]}, qs0, 0)

                # ---- phase 2: remaining q-chunks, ACT-paced; next q-proj
                # and previous normalize+out-proj run as PE/DVE filler,
                # offset to interleave mid-stream instead of piling up at
                # the qt boundary ----
                for qt in range(1, NSC):
                    qs = bass.ts(qt, 512)
                    if qt + 1 < NSC:
                        with tc.high_priority(offset=-100):
                            xt_q = load_xt(16 + qt, bass.ts(qt + 1, 512))
                            rope_proj(xt_q, bass.ts(qt + 1, 512), "q")
                    pso = [
                        PSO.tile([HD + 1, 512], F32, tag=f"o{h}", name=f"o{qt}_{h}")
                        for h in range(HPC)
                    ]
                    pts = {}
                    for kc in range(NKC):
                        off = "dve" if (kc % 6 == 3 and kc < 28) else False
                        pts[kc] = scores_exp(kc, qs, qt, dve_exp=off)
                        if kc == 2 and pend is not None:
                            # flush the previous q-chunk's trailing PVs,
                            # evacuation and normalize/out-proj AFTER this
                            # chunk's first scores so ScalarE never waits
                            # at the boundary
                            p_pso, p_pts, p_qs, p_qt = pend
                            for k2 in sorted(p_pts):
                                attn_pv(p_pso, k2, p_pts[k2])
                            evac(p_pso, p_qs, p_qt)
                            with tc.high_priority(offset=-110):
                                normalize_project(p_qs, p_qt)
                            pend = None
                        if kc >= 3:
                            attn_pv(pso, kc - 3, pts.pop(kc - 3))
                    pend = (pso, pts, qs, qt)
                p_pso, p_pts, p_qs, p_qt = pend
                for k2 in sorted(p_pts):
                    attn_pv(p_pso, k2, p_pts[k2])
                evac(p_pso, p_qs, p_qt)
                normalize_project(p_qs, p_qt)

    return nc


def _rope_tables():
    freqs = 10000.0 ** (-np.linspace(0.0, 1.0, HALF, endpoint=False))
    theta = np.arange(S, dtype=np.float64)[None, :] * freqs[:, None]  # [32, S]
    cos32 = np.cos(theta)
    sin32 = np.sin(theta)
    cosf = np.tile(np.concatenate([cos32, cos32], axis=0), (HPC, 1))
    sinf = np.tile(np.concatenate([-sin32, sin32], axis=0), (HPC, 1))
    return cosf.astype(np.float32), sinf.astype(np.float32)


def kernel(x, wq_k, wq_b, wk_k, wk_b, wv_k, wv_b, wo_k, wo_b):
    from concourse.bass_utils import run_bass_kernel_spmd

    x = np.asarray(x, np.float32)
    wq_k = np.asarray(wq_k, np.float32)
    wq_b = np.asarray(wq_b, np.float32)
    wk_k = np.asarray(wk_k, np.float32)
    wk_b = np.asarray(wk_b, np.float32)
    wv_k = np.asarray(wv_k, np.float32)
    wv_b = np.asarray(wv_b, np.float32)
    wo_k = np.asarray(wo_k, np.float32)
    wo_b = np.asarray(wo_b, np.float32)

    qk_bias = bool(np.any(wq_b) or np.any(wk_b))
    v_bias = bool(np.any(wv_b))
    use_bf16 = os.environ.get("ATTN_MM_DTYPE", "bf16") != "f32r"

    key = (qk_bias, v_bias, use_bf16)
    if key not in _CACHE:
        nc = _build(qk_bias, v_bias, use_bf16)
        _split_multiwait_drains(nc)
        _CACHE[key] = nc
    nc = _CACHE[key]
    import ml_dtypes

    mmdt = ml_dtypes.bfloat16 if use_bf16 else np.float32

    cosf, sinf = _rope_tables()
    perm = np.r_[HALF:HD, 0:HALF]

    in_maps = []
    for c in range(NCORES):
        b = c // 4
        h0 = HPC * (c % 4)
        hsl = slice(h0, h0 + HPC)
        m = {
            "xt": np.ascontiguousarray(x[b].T).astype(mmdt),
            "wq": np.ascontiguousarray(wq_k[:, hsl, :].reshape(DIM, DPC)).astype(mmdt),
            "wqp": np.ascontiguousarray(wq_k[:, hsl, perm].reshape(DIM, DPC)).astype(mmdt),
            "wk": np.ascontiguousarray(wk_k[:, hsl, :].reshape(DIM, DPC)).astype(mmdt),
            "wkp": np.ascontiguousarray(wk_k[:, hsl, perm].reshape(DIM, DPC)).astype(mmdt),
            "wv": np.ascontiguousarray(wv_k[:, hsl, :].reshape(DIM, DPC)).astype(mmdt),
            "wo": np.ascontiguousarray(wo_k[hsl].reshape(DPC, DIM)).astype(mmdt),
            "cosf": cosf,
            "sinf": sinf,
        }
        if qk_bias:
            m["qb"] = np.ascontiguousarray(wq_b[hsl].reshape(DPC, 1))
            m["qbp"] = np.ascontiguousarray(wq_b[hsl][:, perm].reshape(DPC, 1))
            m["kb"] = np.ascontiguousarray(wk_b[hsl].reshape(DPC, 1))
            m["kbp"] = np.ascontiguousarray(wk_b[hsl][:, perm].reshape(DPC, 1))
        if v_bias:
            m["vb"] = np.ascontiguousarray(wv_b[hsl].reshape(1, DPC))
        in_maps.append(m)

    res = run_bass_kernel_spmd(nc, in_maps, list(range(NCORES)))

    out = np.zeros((B, S, DIM), np.float32)
    for c in range(NCORES):
        out[c // 4] += res.results[c]["out"]
    out += wo_b[None, None, :]
    return out


# revision 31
# speedup vs baseline: 1.1645x; 1.1645x over previous
"""Trainium2 Bass kernel for nn_Attention_90074054132266.

Full multi-head attention (B=2, S=4096, D=512, H=8, HD=64) with RoPE on
q/k, sharded over 8 NeuronCores: batch x head-pair (data parallel over
batch, tensor parallel over heads; core c handles batch c//4, heads
2*(c%4), 2*(c%4)+1). Each core computes a partial output projection
(its 2 heads' contribution); the host sums the 4 per-batch partials
(the "all-reduce") and adds wo_b.

Per-core device algorithm (everything stored transposed, f32/bf16):
  - host passes x[b].T, so projections q^T/k^T = wq^T-chunks @ x^T run
    as N=512 matmuls.
  - RoPE via duplicated projections with half-swapped weight columns
    (q2^T[d] = q^T[(d+32)%64 per head]) + sign-baked cos/sin tables:
    q_rot = q^T * cosf + q2^T * sinf  (3 VectorE tensor-tensor ops).
  - scores computed transposed per 128-k-chunk for BOTH heads into one
    PSUM tile S^T[k, (h, q)] = [128, 1024]; exp(S/8) on ScalarE out of
    PSUM (scale folded into the activation; scores ~ N(0,1), exp safe).
  - P@V accumulates per-head O'^T[65, 512] over the 32 k-chunks; V' has
    a ones column appended per head: row 64 = softmax denominator Z.
  - Z path: copy Z rows, reciprocal, bounce via DRAM into a partition-
    broadcast tile izb[128, 512] (rows 0:64 = 1/Z_h0, 64:128 = 1/Z_h1);
    O' evacuated as O'*(1/Z) into a PACKED ot[128, S] (h0 rows 0:64,
    h1 rows 64:128).  No Z transpose, no post-projection scaling.
  - output projection is then a SINGLE matmul per 128-row tile:
    U = ot[:, us].T @ wo (contraction covers both heads), evac + DMA.
  - k/v projection is interleaved with qt=0's attention (chunk sc feeds
    k-chunks 4sc..4sc+3) so ScalarE starts exp'ing ~5us into the kernel
    instead of after the full projection phase.
"""

import os
import sys

sys.path.insert(0, "/opt/trn_rl_repo")

import numpy as np

B, S, DIM, HEADS, HD = 2, 4096, 512, 8, 64
HALF = HD // 2
NCORES = 8
HPC = 2  # heads per core
DPC = HPC * HD  # 128 projection columns per core
NSC = S // 512  # 8 q-column chunks of 512
NKC = S // 128  # 32 k-chunks of 128
NUT = S // 128  # 32 q-row tiles of 128

_CACHE = {}


def _split_multiwait_drains(nc):
    """The walrus build in this container rejects any instruction with
    more than one sync-wait ("Too many sync wait commands"). Hoist the
    extra waits onto preceding same-engine NoOps, leaving one wait on
    the original instruction."""
    import bass_rust
    import concourse.mybir as mybir

    for fn in nc.m.functions:
        for bb in fn.blocks:
            new_insts = []
            changed = False
            for inst in bb.instructions:
                si = getattr(inst, "sync_info", None)
                if si is not None and len(si.on_wait) > 1:
                    waits = list(si.on_wait)
                    for k, w in enumerate(waits[:-1]):
                        d = mybir.InstNoOp(name=f"{inst.name}w{k}", ins=[], outs=[])
                        d.engine = inst.engine
                        d.sync_info = bass_rust.SyncInfo(on_wait=[w], on_update=[])
                        new_insts.append(d)
                    inst.sync_info = bass_rust.SyncInfo(
                        on_wait=[waits[-1]], on_update=list(si.on_update)
                    )
                    changed = True
                new_insts.append(inst)
            if changed:
                bb.instructions = new_insts


def _build(qk_bias, v_bias, use_bf16=True):
    import concourse.bass as bass
    import concourse.tile as tile
    from concourse import mybir

    F32 = mybir.dt.float32
    F32R = mybir.dt.float32r
    BF16 = mybir.dt.bfloat16
    MMD = BF16 if use_bf16 else F32R   # matmul operand dtype (SBUF tiles)
    MME = BF16 if use_bf16 else F32    # DRAM dtype for matmul inputs
    EXP = mybir.ActivationFunctionType.Exp
    MUL = mybir.AluOpType.mult
    ADD = mybir.AluOpType.add
    I16 = mybir.dt.int16
    # Schraudolph fast-exp in bf16: bitcast(int16(A*x + B)) ~= exp(x/8)
    # (max rel err ~3%; used on a few tiles per q-chunk to offload the
    # saturated ScalarE onto the idle VectorE)
    SCH_A = float(2.0**7 / np.log(2.0) * 0.125)
    SCH_B = float(127.0 * 2.0**7 - 366393.0 / 65536.0)

    nc = bass.Bass("TRN2")

    xt_e = nc.declare_dram_parameter("xt", [DIM, S], MME, isOutput=False)
    w_e = {}
    for name in ("wq", "wqp", "wk", "wkp", "wv"):
        w_e[name] = nc.declare_dram_parameter(name, [DIM, DPC], MME, isOutput=False)
    wo_e = nc.declare_dram_parameter("wo", [DPC, DIM], MME, isOutput=False)
    cos_e = nc.declare_dram_parameter("cosf", [DPC, S], F32, isOutput=False)
    sin_e = nc.declare_dram_parameter("sinf", [DPC, S], F32, isOutput=False)
    b_e = {}
    if qk_bias:
        for name in ("qb", "qbp", "kb", "kbp"):
            b_e[name] = nc.declare_dram_parameter(name, [DPC, 1], F32, isOutput=False)
    if v_bias:
        b_e["vb"] = nc.declare_dram_parameter("vb", [1, DPC], F32, isOutput=False)
    out_e = nc.declare_dram_parameter("out", [S, DIM], F32, isOutput=True)

    with tile.TileContext(nc) as tc:
        with (
            tc.tile_pool(name="persist", bufs=1) as P,
            tc.tile_pool(name="work", bufs=2) as W,
            tc.tile_pool(name="ptp", bufs=8) as PT,
        ):
            # ---- persistent SBUF tensors ----
            qr = P.tile([DPC, S], MMD, tag="qr")  # rotated q^T
            # rotated k^T, zero-padded per head to full K=128 contraction
            # (row-masked K=64 matmuls don't count as PE-busy for the HAM
            # clock gate; mixing them with PV pins the PE at 1.2 GHz)
            krA = P.tile([DPC, S], MMD, tag="krA")
            krB = P.tile([DPC, S], MMD, tag="krB")
            # Z rows staging (head h at partition 32h; DVE outputs must be
            # quadrant-aligned)
            zrow = P.tile([33, S], F32, tag="zrow")
            # V' rows: per k-chunk st, V[k, :] for head A cols 0:64 + ones
            # col 64, head B cols 65:129 + ones col 129.
            VW = 2 * (HD + 1)  # 130
            vb_sb = P.tile([128, NKC, VW], MMD, tag="vboth")
            # normalized O^T packed: rows 0:64 head A, 64:128 head B
            ot = P.tile([DPC, S], MMD, tag="ot")
            wo_sb = P.tile([DPC, DIM], MMD, tag="wo")


            bias_sb = {}
            if qk_bias:
                for name in ("qb", "qbp", "kb", "kbp"):
                    t = P.tile([DPC, 1], F32, tag=name)
                    nc.sync.dma_start(out=t, in_=b_e[name][:])
                    bias_sb[name] = t
            if v_bias:
                vbias_bc = P.tile([128, DPC], F32, tag="vbias")
                src = bass.AP(
                    tensor=b_e["vb"].tensor,
                    offset=b_e["vb"].offset,
                    ap=[[0, 128], [1, DPC]],
                )
                nc.sync.dma_start(out=vbias_bc, in_=src)

            # ---- PSUM budget (8 banks): scores 2x2 + pso 2x1 + proj 1
            # + out-proj U 1 (separate pools so filler rotations decouple)
            with (
                tc.tile_pool(name="xtp", bufs=2) as XT,
                tc.tile_pool(name="wpool", bufs=1) as WP,
                tc.tile_pool(name="psp", bufs=1, space="PSUM") as PSP,
                tc.tile_pool(name="psu", bufs=1, space="PSUM") as PSU,
                tc.tile_pool(name="pss", bufs=2, space="PSUM") as PSS,
                tc.tile_pool(name="pso", bufs=1, space="PSUM") as PSO,
            ):
                cos_sb = WP.tile([DPC, S], F32, tag="cos")
                sin_sb = WP.tile([DPC, S], F32, tag="sin")

                def load_tables(sc):
                    qs_ = bass.ts(sc, 512)
                    nc.sync.dma_start(out=cos_sb[:, qs_], in_=cos_e[:, qs_])
                    nc.sync.dma_start(out=sin_sb[:, qs_], in_=sin_e[:, qs_])

                w_sb = {}

                def load_w(name):
                    t = WP.tile([128, 4, DPC], MMD, tag=name)
                    nc.sync.dma_start(
                        out=t,
                        in_=(
                            w_e[name][:].rearrange("(c p) m -> p c m", p=128)
                            if use_bf16
                            else w_e[name][:]
                            .rearrange("(c p) m -> p c m", p=128)
                            .bitcast(F32R)
                        ),
                    )
                    w_sb[name] = t

                xt_r = xt_e[:].rearrange("(c p) s -> c p s", p=128)

                def load_xt(sc, qs):
                    xt_c = []
                    for c in range(4):
                        t = XT.tile(
                            [128, 512], MMD, tag=f"xt{c}", name=f"xt{c}_{sc}"
                        )
                        nc.sync.dma_start(
                            out=t,
                            in_=xt_r[c, :, qs]
                            if use_bf16
                            else xt_r[c, :, qs].bitcast(F32R),
                        )
                        xt_c.append(t)
                    return xt_c

                def rope_proj(xt_c, qs, which, pools=None):
                    pools = pools or (PSP, PSP)
                    # one 512-col chunk of rotated q^T or (split) k^T
                    wn, wpn, bn, bpn = (
                        ("wq", "wqp", "qb", "qbp")
                        if which == "q"
                        else ("wk", "wkp", "kb", "kbp")
                    )
                    ps1 = pools[0].tile(
                        [128, 512], F32, tag=pools[0].name[-3:], name=f"p1_{which}{qs}"
                    )
                    for c in range(4):
                        nc.tensor.matmul(
                            ps1,
                            w_sb[wn][:, c, :],
                            xt_c[c][:],
                            start=(c == 0),
                            stop=(c == 3),
                        )
                    if qk_bias:
                        s1 = W.tile([128, 512], F32, tag="rope1")
                        nc.vector.tensor_scalar_add(s1, ps1, bias_sb[bn])
                    else:
                        s1 = ps1
                    t3 = W.tile([128, 512], F32, tag="rope3")
                    nc.vector.tensor_tensor(out=t3, in0=s1, in1=cos_sb[:, qs], op=MUL)
                    ps2 = pools[1].tile(
                        [128, 512], F32, tag=pools[1].name[-3:], name=f"p2_{which}{qs}"
                    )
                    for c in range(4):
                        nc.tensor.matmul(
                            ps2,
                            w_sb[wpn][:, c, :],
                            xt_c[c][:],
                            start=(c == 0),
                            stop=(c == 3),
                        )
                    if qk_bias:
                        s2 = W.tile([128, 512], F32, tag="rope2")
                        nc.vector.tensor_scalar_add(s2, ps2, bias_sb[bpn])
                    else:
                        s2 = ps2
                    t4 = W.tile([128, 512], F32, tag="rope4")
                    nc.vector.tensor_tensor(out=t4, in0=s2, in1=sin_sb[:, qs], op=MUL)
                    if which == "q":
                        nc.vector.tensor_tensor(out=qr[:, qs], in0=t3, in1=t4, op=ADD)
                    else:
                        nc.vector.tensor_tensor(
                            out=krA[0:HD, qs], in0=t3[0:HD, :], in1=t4[0:HD, :], op=ADD
                        )
                        nc.vector.tensor_tensor(
                            out=krB[HD:DPC, qs],
                            in0=t3[HD:DPC, :],
                            in1=t4[HD:DPC, :],
                            op=ADD,
                        )

                def v_proj_part(xt_c, sc, stl, pl):
                    st = sc * 4 + stl
                    psv = pl.tile([128, 512], F32, tag=pl.name[-3:], name=f"pv{st}")
                    for c in range(4):
                        nc.tensor.matmul(
                            psv[:, 0:128],
                            xt_c[c][:, bass.ts(stl, 128)],
                            w_sb["wv"][:, c, :],
                            start=(c == 0),
                            stop=(c == 3),
                        )
                    dsts = vb_sb[:, st, :].rearrange(
                        "p (j w) -> p j w", w=HD + 1
                    )[:, :, 0:HD]
                    if v_bias:
                        nc.vector.tensor_tensor(
                            out=dsts, in0=psv[:, 0:128], in1=vbias_bc, op=ADD
                        )
                    else:
                        nc.vector.tensor_copy(out=dsts, in_=psv[:, 0:128])

                # DRAM scratch for the 1/Z partition-broadcast bounce
                zscr = nc.dram_tensor("zscr", [HPC, S], F32)
                zscr_ap = zscr[:]

                def scores_exp(kc, qs, qt, dve_exp=False):
                    # scores for both heads of k-chunk kc, then exp
                    pss_t = PSS.tile([128, 1024], F32, tag="s", name=f"ss{qt}_{kc}")
                    nc.tensor.matmul(
                        pss_t[:, 0:512],
                        krA[:, bass.ts(kc, 128)],
                        qr[:, qs],
                        start=True,
                        stop=True,
                    )
                    nc.tensor.matmul(
                        pss_t[:, 512:1024],
                        krB[:, bass.ts(kc, 128)],
                        qr[:, qs],
                        start=True,
                        stop=True,
                    )
                    if dve_exp:
                        pti = PT.tile(
                            [128, 1024], I16, tag="pti", name=f"pti{qt}_{kc}"
                        )
                        eng = nc.gpsimd if dve_exp == "gps" else nc.vector
                        eng.tensor_scalar(
                            out=pti,
                            in0=pss_t,
                            scalar1=SCH_A,
                            scalar2=SCH_B,
                            op0=MUL,
                            op1=ADD,
                        )
                        return pti[:].bitcast(BF16)
                    pt = PT.tile([128, 1024], MMD, tag="pt", name=f"pt{qt}_{kc}")
                    nc.scalar.activation(out=pt, in_=pss_t, func=EXP, scale=0.125)
                    return pt

                def attn_pv(pso, kc, pt):
                    for h in range(HPC):
                        nc.tensor.matmul(
                            pso[h],
                            vb_sb[:, kc, bass.ds(h * (HD + 1), HD + 1)],
                            pt[:, bass.ts(h, 512)],
                            start=(kc == 0),
                            stop=(kc == NKC - 1),
                        )

                def attn_kc(pso, kc, qs, qt):
                    attn_pv(pso, kc, scores_exp(kc, qs, qt))

                def evac(pso, qs, qt):
                    # fast PSUM evacuation: unnormalized O' into packed ot,
                    # Z rows into zrow; releases the pso banks quickly so
                    # the next q-chunk's PV never stalls.
                    for h in range(HPC):
                        nc.vector.tensor_copy(
                            out=ot[h * HD : (h + 1) * HD, qs], in_=pso[h][0:HD, :]
                        )
                        nc.vector.tensor_copy(
                            out=zrow[32 * h : 32 * h + 1, qs],
                            in_=pso[h][HD : HD + 1, :],
                        )

                def normalize_project(qs, qt, stagger=False):
                    # Z -> DRAM -> partition-broadcast -> 1/Z, normalize ot
                    # in place, then the fused single-matmul out-projection.
                    # Latency-tolerant: runs as filler during the next qt.
                    for h in range(HPC):
                        nc.sync.dma_start(
                            out=zscr[h : h + 1, qs],
                            in_=zrow[32 * h : 32 * h + 1, qs],
                        )
                    izbz = W.tile([128, 512], F32, tag="izbz", name=f"izbz{qt}")
                    for h in range(HPC):
                        zsl = zscr_ap[h : h + 1, qs]
                        src = bass.AP(
                            tensor=zsl.tensor,
                            offset=zsl.offset,
                            ap=[[0, HD], [1, 512]],
                        )
                        nc.sync.dma_start(out=izbz[h * HD : (h + 1) * HD, :], in_=src)
                    izb = W.tile([128, 512], F32, tag="izb", name=f"izb{qt}")
                    for utl in range(4):
                        stk = (
                            tc.high_priority(offset=-(15 + 36 * utl))
                            if stagger
                            else None
                        )
                        if stk is not None:
                            stk.__enter__()
                        ut = qt * 4 + utl
                        us = bass.ts(ut, 128)
                        cs = bass.ts(utl, 128)
                        nc.vector.reciprocal(out=izb[:, cs], in_=izbz[:, cs])
                        for h in range(HPC):
                            otb = ot[h * HD : (h + 1) * HD, qs].rearrange(
                                "p (u c) -> p u c", c=128
                            )[:, utl, :]
                            nc.vector.tensor_tensor(
                                out=otb,
                                in0=otb,
                                in1=izb[h * HD : (h + 1) * HD, cs],
                                op=MUL,
                            )
                        psu = PSU.tile(
                            [128, DIM], F32, tag=PSU.name[-3:], name=f"u{ut}"
                        )
                        nc.tensor.matmul(
                            psu, ot[:, us], wo_sb[:, :], start=True, stop=True
                        )
                        t_out = W.tile([128, DIM], F32, tag="uout")
                        nc.vector.tensor_copy(out=t_out, in_=psu)
                        nc.sync.dma_start(out=out_e[us, :], in_=t_out)
                        if stk is not None:
                            stk.__exit__(None, None, None)

                # ---- phase 1: q0 proj, then k/v proj interleaved with
                # qt=0's attention (chunk sc feeds k-chunks 4sc..4sc+3).
                # DMAs are emitted in dependency order so q0/kv0 data
                # arrives first; kv-proj PSUM temps alternate the two
                # spare banks (PSP/PSU) to pipeline matmul vs. evac. ----
                load_tables(0)
                load_w("wq")
                load_w("wqp")
                xt_c = load_xt(0, bass.ts(0, 512))
                load_w("wk")
                load_w("wkp")
                load_w("wv")
                rope_proj(xt_c, bass.ts(0, 512), "q", pools=(PSP, PSU))
                qs0 = bass.ts(0, 512)
                pso = [
                    PSO.tile([HD + 1, 512], F32, tag=f"o{h}", name=f"o0_{h}")
                    for h in range(HPC)
                ]
                for sc in range(NSC):
                    qs = bass.ts(sc, 512)
                    if sc >= 1:
                        load_tables(sc)
                    xt_c = load_xt(8 + sc, qs)
                    # zero-pad rows of this chunk only (a full-tensor
                    # memset would hold up the first scores for ~16us)
                    nc.vector.memset(krA[HD:DPC, qs], 0.0)
                    nc.vector.memset(krB[0:HD, qs], 0.0)
                    ones_sc = vb_sb[:, bass.ds(sc * 4, 4), :].rearrange(
                        "p s (j w) -> p s j w", w=HD + 1
                    )[:, :, :, HD : HD + 1]
                    nc.vector.memset(
                        ones_sc if use_bf16 else ones_sc.bitcast(F32), 1.0
                    )
                    rope_proj(xt_c, qs, "k", pools=(PSP, PSU))
                    pts = []
                    for kcl in range(4):
                        pts.append(scores_exp(sc * 4 + kcl, qs0, 0))
                        v_proj_part(xt_c, sc, kcl, (PSP, PSU)[kcl % 2])
                        if kcl >= 1:
                            attn_pv(pso, sc * 4 + kcl - 1, pts[kcl - 1])
                    if sc < NSC - 1:
                        attn_pv(pso, sc * 4 + 3, pts[3])
                    if sc == 6:
                        # project q1 so qt=1 can start right after qt=0
                        # (normal priority; a mid-loop high_priority insert
                        # inverts proj pool rotation order and deadlocks)
                        xt_q = load_xt(16, bass.ts(1, 512))
                        rope_proj(xt_q, bass.ts(1, 512), "q", pools=(PSP, PSU))
                nc.sync.dma_start(
                    out=wo_sb, in_=wo_e[:] if use_bf16 else wo_e[:].bitcast(F32R)
                )
                pend = (pso, {NKC - 1: pts[3]}, qs0, 0)

                # ---- phase 2: remaining q-chunks, ACT-paced; next q-proj
                # and previous normalize+out-proj run as PE/DVE filler,
                # offset to interleave mid-stream instead of piling up at
                # the qt boundary ----
                for qt in range(1, NSC):
                    qs = bass.ts(qt, 512)
                    if qt + 1 < NSC:
                        with tc.high_priority(offset=-100):
                            xt_q = load_xt(16 + qt, bass.ts(qt + 1, 512))
                            rope_proj(xt_q, bass.ts(qt + 1, 512), "q")
                    pso = [
                        PSO.tile([HD + 1, 512], F32, tag=f"o{h}", name=f"o{qt}_{h}")
                        for h in range(HPC)
                    ]
                    pts = {}
                    for kc in range(NKC):
                        off = "dve" if (kc % 6 == 3 and kc < 28) else False
                        pts[kc] = scores_exp(kc, qs, qt, dve_exp=off)
                        if kc == 2 and pend is not None:
                            # flush the previous q-chunk's trailing PVs,
                            # evacuation and normalize/out-proj AFTER this
                            # chunk's first scores so ScalarE never waits
                            # at the boundary
                            p_pso, p_pts, p_qs, p_qt = pend
                            for k2 in sorted(p_pts):
                                attn_pv(p_pso, k2, p_pts[k2])
                            evac(p_pso, p_qs, p_qt)
                            with tc.high_priority(offset=-110):
                                normalize_project(p_qs, p_qt)
                            pend = None
                        if kc >= 3:
                            attn_pv(pso, kc - 3, pts.pop(kc - 3))
                    pend = (pso, pts, qs, qt)
                p_pso, p_pts, p_qs, p_qt = pend
                for k2 in sorted(p_pts):
                    attn_pv(p_pso, k2, p_pts[k2])
                evac(p_pso, p_qs, p_qt)
                normalize_project(p_qs, p_qt)

    return nc


def _rope_tables():
    freqs = 10000.0 ** (-np.linspace(0.0, 1.0, HALF, endpoint=False))
    theta = np.arange(S, dtype=np.float64)[None, :] * freqs[:, None]  # [32, S]
    cos32 = np.cos(theta)
    sin32 = np.sin(theta)
    cosf = np.tile(np.concatenate([cos32, cos32], axis=0), (HPC, 1))
    sinf = np.tile(np.concatenate([-sin32, sin32], axis=0), (HPC, 1))
    return cosf.astype(np.float32), sinf.astype(np.float32)


def kernel(x, wq_k, wq_b, wk_k, wk_b, wv_k, wv_b, wo_k, wo_b):
    from concourse.bass_utils import run_bass_kernel_spmd

    x = np.asarray(x, np.float32)
    wq_k = np.asarray(wq_k, np.float32)
    wq_b = np.asarray(wq_b, np.float32)
    wk_k = np.asarray(wk_k, np.float32)
    wk_b = np.asarray(wk_b, np.float32)
    wv_k = np.asarray(wv_k, np.float32)
    wv_b = np.asarray(wv_b, np.float32)
    wo_k = np.asarray(wo_k, np.float32)
    wo_b = np.asarray(wo_b, np.float32)

    qk_bias = bool(np.any(wq_b) or np.any(wk_b))
    v_bias = bool(np.any(wv_b))
    use_bf16 = os.environ.get("ATTN_MM_DTYPE", "bf16") != "f32r"

    key = (qk_bias, v_bias, use_bf16)
    if key not in _CACHE:
        nc = _build(qk_bias, v_bias, use_bf16)
        _split_multiwait_drains(nc)
        _CACHE[key] = nc
    nc = _CACHE[key]
    import ml_dtypes

    mmdt = ml_dtypes.bfloat16 if use_bf16 else np.float32

    cosf, sinf = _rope_tables()
    perm = np.r_[HALF:HD, 0:HALF]

    in_maps = []
    for c in range(NCORES):
        b = c // 4
        h0 = HPC * (c % 4)
        hsl = slice(h0, h0 + HPC)
        m = {
            "xt": np.ascontiguousarray(x[b].T).astype(mmdt),
            "wq": np.ascontiguousarray(wq_k[:, hsl, :].reshape(DIM, DPC)).astype(mmdt),
            "wqp": np.ascontiguousarray(wq_k[:, hsl, perm].reshape(DIM, DPC)).astype(mmdt),
            "wk": np.ascontiguousarray(wk_k[:, hsl, :].reshape(DIM, DPC)).astype(mmdt),
            "wkp": np.ascontiguousarray(wk_k[:, hsl, perm].reshape(DIM, DPC)).astype(mmdt),
            "wv": np.ascontiguousarray(wv_k[:, hsl, :].reshape(DIM, DPC)).astype(mmdt),
            "wo": np.ascontiguousarray(wo_k[hsl].reshape(DPC, DIM)).astype(mmdt),
            "cosf": cosf,
            "sinf": sinf,
        }
        if qk_bias:
            m["qb"] = np.ascontiguousarray(wq_b[hsl].reshape(DPC, 1))
            m["qbp"] = np.ascontiguousarray(wq_b[hsl][:, perm].reshape(DPC, 1))
            m["kb"] = np.ascontiguousarray(wk_b[hsl].reshape(DPC, 1))
            m["kbp"] = np.ascontiguousarray(wk_b[hsl][:, perm].reshape(DPC, 1))
        if v_bias:
            m["vb"] = np.ascontiguousarray(wv_b[hsl].reshape(1, DPC))
        in_maps.append(m)

    res = run_bass_kernel_spmd(nc, in_maps, list(range(NCORES)))

    out = np.zeros((B, S, DIM), np.float32)
    for c in range(NCORES):
        out[c // 4] += res.results[c]["out"]
    out += wo_b[None, None, :]
    return out


# revision 32
# speedup vs baseline: 1.1757x; 1.0096x over previous
"""Trainium2 Bass kernel for nn_Attention_90074054132266.

Full multi-head attention (B=2, S=4096, D=512, H=8, HD=64) with RoPE on
q/k, sharded over 8 NeuronCores: batch x head-pair (data parallel over
batch, tensor parallel over heads; core c handles batch c//4, heads
2*(c%4), 2*(c%4)+1). Each core computes a partial output projection
(its 2 heads' contribution); the host sums the 4 per-batch partials
(the "all-reduce") and adds wo_b.

Per-core device algorithm (everything stored transposed, f32/bf16):
  - host passes x[b].T, so projections q^T/k^T = wq^T-chunks @ x^T run
    as N=512 matmuls.
  - RoPE via duplicated projections with half-swapped weight columns
    (q2^T[d] = q^T[(d+32)%64 per head]) + sign-baked cos/sin tables.
  - scores per 128-k-chunk for BOTH heads in one PSUM tile
    S^T[k, (h, q)] = [128, 1024]; exp(S/8) on ScalarE out of PSUM
    (scale folded into the activation; no max subtraction: scores are
    ~N(0,1), exp is safe in fp32).  5 of 32 k-chunks per q-chunk use a
    VectorE int16-Schraudolph fast exp (bitcast(int16(A*x+B)) in bf16,
    ~3%% max element err) to offload the saturated ScalarE.
  - P@V accumulates per-head O'^T[65, 512] over 32 k-chunks; V' has a
    ones column per head: row 64 = softmax denominator Z for free.
    PVs trail their exp by 3 k-chunks so a slow consumer never stalls
    the PE's static instruction order.
  - Z path (off the PSUM-release path): evac O'/Z with plain copies,
    then Z -> DRAM -> partition-broadcast izb[128, 512] -> reciprocal,
    normalize ot in place, fused single-matmul out-projection per
    128-row tile (U = ot.T @ wo contracts both heads at once).  All of
    it runs as low-priority filler during the next q-chunk.
  - schedule: k/v projection is interleaved with qt=0's attention
    (chunk sc feeds k-chunks 4sc..4sc+3); each later q-chunk flushes
    the previous chunk's trailing PVs/evac/normalize AFTER its first
    scores so ScalarE never idles at chunk boundaries; next q-chunk's
    q-projection runs as PE filler at a small priority offset so it
    interleaves mid-stream.
  - PSUM budget (8 banks): scores 2x2 + O' 2x1 + projection 1 +
    out-projection 1.  Filler pools are separate so buffer-rotation
    WARs cannot invert the priority order (that deadlocks the queues).
"""

import os
import sys

sys.path.insert(0, "/opt/trn_rl_repo")

import numpy as np

B, S, DIM, HEADS, HD = 2, 4096, 512, 8, 64
HALF = HD // 2
NCORES = 8
HPC = 2  # heads per core
DPC = HPC * HD  # 128 projection columns per core
NSC = S // 512  # 8 q-column chunks of 512
NKC = S // 128  # 32 k-chunks of 128
NUT = S // 128  # 32 q-row tiles of 128

_CACHE = {}


def _split_multiwait_drains(nc):
    """The walrus build in this container rejects any instruction with
    more than one sync-wait ("Too many sync wait commands"). Hoist the
    extra waits onto preceding same-engine NoOps, leaving one wait on
    the original instruction."""
    import bass_rust
    import concourse.mybir as mybir

    for fn in nc.m.functions:
        for bb in fn.blocks:
            new_insts = []
            changed = False
            for inst in bb.instructions:
                si = getattr(inst, "sync_info", None)
                if si is not None and len(si.on_wait) > 1:
                    waits = list(si.on_wait)
                    for k, w in enumerate(waits[:-1]):
                        d = mybir.InstNoOp(name=f"{inst.name}w{k}", ins=[], outs=[])
                        d.engine = inst.engine
                        d.sync_info = bass_rust.SyncInfo(on_wait=[w], on_update=[])
                        new_insts.append(d)
                    inst.sync_info = bass_rust.SyncInfo(
                        on_wait=[waits[-1]], on_update=list(si.on_update)
                    )
                    changed = True
                new_insts.append(inst)
            if changed:
                bb.instructions = new_insts


def _build(qk_bias, v_bias, use_bf16=True):
    import concourse.bass as bass
    import concourse.tile as tile
    from concourse import mybir

    F32 = mybir.dt.float32
    F32R = mybir.dt.float32r
    BF16 = mybir.dt.bfloat16
    MMD = BF16 if use_bf16 else F32R   # matmul operand dtype (SBUF tiles)
    MME = BF16 if use_bf16 else F32    # DRAM dtype for matmul inputs
    EXP = mybir.ActivationFunctionType.Exp
    MUL = mybir.AluOpType.mult
    ADD = mybir.AluOpType.add
    I16 = mybir.dt.int16
    # Schraudolph fast-exp in bf16: bitcast(int16(A*x + B)) ~= exp(x/8)
    # (max rel err ~3%; used on a few tiles per q-chunk to offload the
    # saturated ScalarE onto the idle VectorE)
    SCH_A = float(2.0**7 / np.log(2.0) * 0.125)
    SCH_B = float(127.0 * 2.0**7 - 366393.0 / 65536.0)

    nc = bass.Bass("TRN2")

    xt_e = nc.declare_dram_parameter("xt", [DIM, S], MME, isOutput=False)
    w_e = {}
    for name in ("wq", "wqp", "wk", "wkp", "wv"):
        w_e[name] = nc.declare_dram_parameter(name, [DIM, DPC], MME, isOutput=False)
    wo_e = nc.declare_dram_parameter("wo", [DPC, DIM], MME, isOutput=False)
    cos_e = nc.declare_dram_parameter("cosf", [DPC, S], F32, isOutput=False)
    sin_e = nc.declare_dram_parameter("sinf", [DPC, S], F32, isOutput=False)
    b_e = {}
    if qk_bias:
        for name in ("qb", "qbp", "kb", "kbp"):
            b_e[name] = nc.declare_dram_parameter(name, [DPC, 1], F32, isOutput=False)
    if v_bias:
        b_e["vb"] = nc.declare_dram_parameter("vb", [1, DPC], F32, isOutput=False)
    out_e = nc.declare_dram_parameter("out", [S, DIM], F32, isOutput=True)

    with tile.TileContext(nc) as tc:
        with (
            tc.tile_pool(name="persist", bufs=1) as P,
            tc.tile_pool(name="work", bufs=2) as W,
            tc.tile_pool(name="ptp", bufs=8) as PT,
        ):
            # ---- persistent SBUF tensors ----
            qr = P.tile([DPC, S], MMD, tag="qr")  # rotated q^T
            # rotated k^T, zero-padded per head to full K=128 contraction
            # (row-masked K=64 matmuls don't count as PE-busy for the HAM
            # clock gate; mixing them with PV pins the PE at 1.2 GHz)
            krA = P.tile([DPC, S], MMD, tag="krA")
            krB = P.tile([DPC, S], MMD, tag="krB")
            # Z rows staging (head h at partition 32h; DVE outputs must be
            # quadrant-aligned)
            zrow = P.tile([33, S], F32, tag="zrow")
            # V' rows: per k-chunk st, V[k, :] for head A cols 0:64 + ones
            # col 64, head B cols 65:129 + ones col 129.
            VW = 2 * (HD + 1)  # 130
            vb_sb = P.tile([128, NKC, VW], MMD, tag="vboth")
            # normalized O^T packed: rows 0:64 head A, 64:128 head B
            ot = P.tile([DPC, S], MMD, tag="ot")
            wo_sb = P.tile([DPC, DIM], MMD, tag="wo")


            bias_sb = {}
            if qk_bias:
                for name in ("qb", "qbp", "kb", "kbp"):
                    t = P.tile([DPC, 1], F32, tag=name)
                    nc.sync.dma_start(out=t, in_=b_e[name][:])
                    bias_sb[name] = t
            if v_bias:
                vbias_bc = P.tile([128, DPC], F32, tag="vbias")
                src = bass.AP(
                    tensor=b_e["vb"].tensor,
                    offset=b_e["vb"].offset,
                    ap=[[0, 128], [1, DPC]],
                )
                nc.sync.dma_start(out=vbias_bc, in_=src)

            # ---- PSUM budget (8 banks): scores 2x2 + pso 2x1 + proj 1
            # + out-proj U 1 (separate pools so filler rotations decouple)
            with (
                tc.tile_pool(name="xtp", bufs=2) as XT,
                tc.tile_pool(name="wpool", bufs=1) as WP,
                tc.tile_pool(name="psp", bufs=1, space="PSUM") as PSP,
                tc.tile_pool(name="psu", bufs=1, space="PSUM") as PSU,
                tc.tile_pool(name="pss", bufs=2, space="PSUM") as PSS,
                tc.tile_pool(name="pso", bufs=1, space="PSUM") as PSO,
            ):
                cos_sb = WP.tile([DPC, S], F32, tag="cos")
                sin_sb = WP.tile([DPC, S], F32, tag="sin")

                def load_tables(sc):
                    qs_ = bass.ts(sc, 512)
                    nc.sync.dma_start(out=cos_sb[:, qs_], in_=cos_e[:, qs_])
                    nc.sync.dma_start(out=sin_sb[:, qs_], in_=sin_e[:, qs_])

                w_sb = {}

                def load_w(name):
                    t = WP.tile([128, 4, DPC], MMD, tag=name)
                    nc.sync.dma_start(
                        out=t,
                        in_=(
                            w_e[name][:].rearrange("(c p) m -> p c m", p=128)
                            if use_bf16
                            else w_e[name][:]
                            .rearrange("(c p) m -> p c m", p=128)
                            .bitcast(F32R)
                        ),
                    )
                    w_sb[name] = t

                xt_r = xt_e[:].rearrange("(c p) s -> c p s", p=128)

                def load_xt(sc, qs):
                    xt_c = []
                    for c in range(4):
                        t = XT.tile(
                            [128, 512], MMD, tag=f"xt{c}", name=f"xt{c}_{sc}"
                        )
                        nc.sync.dma_start(
                            out=t,
                            in_=xt_r[c, :, qs]
                            if use_bf16
                            else xt_r[c, :, qs].bitcast(F32R),
                        )
                        xt_c.append(t)
                    return xt_c

                def rope_proj(xt_c, qs, which, pools=None):
                    pools = pools or (PSP, PSP)
                    # one 512-col chunk of rotated q^T or (split) k^T
                    wn, wpn, bn, bpn = (
                        ("wq", "wqp", "qb", "qbp")
                        if which == "q"
                        else ("wk", "wkp", "kb", "kbp")
                    )
                    ps1 = pools[0].tile(
                        [128, 512], F32, tag=pools[0].name[-3:], name=f"p1_{which}{qs}"
                    )
                    for c in range(4):
                        nc.tensor.matmul(
                            ps1,
                            w_sb[wn][:, c, :],
                            xt_c[c][:],
                            start=(c == 0),
                            stop=(c == 3),
                        )
                    if qk_bias:
                        s1 = W.tile([128, 512], F32, tag="rope1")
                        nc.vector.tensor_scalar_add(s1, ps1, bias_sb[bn])
                    else:
                        s1 = ps1
                    t3 = W.tile([128, 512], F32, tag="rope3")
                    nc.vector.tensor_tensor(out=t3, in0=s1, in1=cos_sb[:, qs], op=MUL)
                    ps2 = pools[1].tile(
                        [128, 512], F32, tag=pools[1].name[-3:], name=f"p2_{which}{qs}"
                    )
                    for c in range(4):
                        nc.tensor.matmul(
                            ps2,
                            w_sb[wpn][:, c, :],
                            xt_c[c][:],
                            start=(c == 0),
                            stop=(c == 3),
                        )
                    if qk_bias:
                        s2 = W.tile([128, 512], F32, tag="rope2")
                        nc.vector.tensor_scalar_add(s2, ps2, bias_sb[bpn])
                    else:
                        s2 = ps2
                    t4 = W.tile([128, 512], F32, tag="rope4")
                    nc.vector.tensor_tensor(out=t4, in0=s2, in1=sin_sb[:, qs], op=MUL)
                    if which == "q":
                        nc.vector.tensor_tensor(out=qr[:, qs], in0=t3, in1=t4, op=ADD)
                    else:
                        nc.vector.tensor_tensor(
                            out=krA[0:HD, qs], in0=t3[0:HD, :], in1=t4[0:HD, :], op=ADD
                        )
                        nc.vector.tensor_tensor(
                            out=krB[HD:DPC, qs],
                            in0=t3[HD:DPC, :],
                            in1=t4[HD:DPC, :],
                            op=ADD,
                        )

                def v_proj_part(xt_c, sc, stl, pl):
                    st = sc * 4 + stl
                    psv = pl.tile([128, 512], F32, tag=pl.name[-3:], name=f"pv{st}")
                    for c in range(4):
                        nc.tensor.matmul(
                            psv[:, 0:128],
                            xt_c[c][:, bass.ts(stl, 128)],
                            w_sb["wv"][:, c, :],
                            start=(c == 0),
                            stop=(c == 3),
                        )
                    dsts = vb_sb[:, st, :].rearrange(
                        "p (j w) -> p j w", w=HD + 1
                    )[:, :, 0:HD]
                    if v_bias:
                        nc.vector.tensor_tensor(
                            out=dsts, in0=psv[:, 0:128], in1=vbias_bc, op=ADD
                        )
                    else:
                        nc.vector.tensor_copy(out=dsts, in_=psv[:, 0:128])

                # DRAM scratch for the 1/Z partition-broadcast bounce
                zscr = nc.dram_tensor("zscr", [HPC, S], F32)
                zscr_ap = zscr[:]

                def scores_exp(kc, qs, qt, dve_exp=False):
                    # scores for both heads of k-chunk kc, then exp
                    pss_t = PSS.tile([128, 1024], F32, tag="s", name=f"ss{qt}_{kc}")
                    nc.tensor.matmul(
                        pss_t[:, 0:512],
                        krA[:, bass.ts(kc, 128)],
                        qr[:, qs],
                        start=True,
                        stop=True,
                    )
                    nc.tensor.matmul(
                        pss_t[:, 512:1024],
                        krB[:, bass.ts(kc, 128)],
                        qr[:, qs],
                        start=True,
                        stop=True,
                    )
                    if dve_exp:
                        pti = PT.tile(
                            [128, 1024], I16, tag="pti", name=f"pti{qt}_{kc}"
                        )
                        eng = nc.gpsimd if dve_exp == "gps" else nc.vector
                        eng.tensor_scalar(
                            out=pti,
                            in0=pss_t,
                            scalar1=SCH_A,
                            scalar2=SCH_B,
                            op0=MUL,
                            op1=ADD,
                        )
                        return pti[:].bitcast(BF16)
                    pt = PT.tile([128, 1024], MMD, tag="pt", name=f"pt{qt}_{kc}")
                    nc.scalar.activation(out=pt, in_=pss_t, func=EXP, scale=0.125)
                    return pt

                def attn_pv(pso, kc, pt):
                    for h in range(HPC):
                        nc.tensor.matmul(
                            pso[h],
                            vb_sb[:, kc, bass.ds(h * (HD + 1), HD + 1)],
                            pt[:, bass.ts(h, 512)],
                            start=(kc == 0),
                            stop=(kc == NKC - 1),
                        )

                def attn_kc(pso, kc, qs, qt):
                    attn_pv(pso, kc, scores_exp(kc, qs, qt))

                def evac(pso, qs, qt):
                    # fast PSUM evacuation: unnormalized O' into packed ot,
                    # Z rows into zrow; releases the pso banks quickly so
                    # the next q-chunk's PV never stalls.
                    for h in range(HPC):
                        nc.vector.tensor_copy(
                            out=ot[h * HD : (h + 1) * HD, qs], in_=pso[h][0:HD, :]
                        )
                        nc.vector.tensor_copy(
                            out=zrow[32 * h : 32 * h + 1, qs],
                            in_=pso[h][HD : HD + 1, :],
                        )

                def normalize_project(qs, qt, stagger=False):
                    # Z -> DRAM -> partition-broadcast -> 1/Z, normalize ot
                    # in place, then the fused single-matmul out-projection.
                    # Latency-tolerant: runs as filler during the next qt.
                    for h in range(HPC):
                        nc.sync.dma_start(
                            out=zscr[h : h + 1, qs],
                            in_=zrow[32 * h : 32 * h + 1, qs],
                        )
                    izbz = W.tile([128, 512], F32, tag="izbz", name=f"izbz{qt}")
                    for h in range(HPC):
                        zsl = zscr_ap[h : h + 1, qs]
                        src = bass.AP(
                            tensor=zsl.tensor,
                            offset=zsl.offset,
                            ap=[[0, HD], [1, 512]],
                        )
                        nc.sync.dma_start(out=izbz[h * HD : (h + 1) * HD, :], in_=src)
                    izb = W.tile([128, 512], F32, tag="izb", name=f"izb{qt}")
                    for utl in range(4):
                        stk = (
                            tc.high_priority(offset=-(15 + 36 * utl))
                            if stagger
                            else None
                        )
                        if stk is not None:
                            stk.__enter__()
                        ut = qt * 4 + utl
                        us = bass.ts(ut, 128)
                        cs = bass.ts(utl, 128)
                        nc.vector.reciprocal(out=izb[:, cs], in_=izbz[:, cs])
                        for h in range(HPC):
                            otb = ot[h * HD : (h + 1) * HD, qs].rearrange(
                                "p (u c) -> p u c", c=128
                            )[:, utl, :]
                            nc.vector.tensor_tensor(
                                out=otb,
                                in0=otb,
                                in1=izb[h * HD : (h + 1) * HD, cs],
                                op=MUL,
                            )
                        psu = PSU.tile(
                            [128, DIM], F32, tag=PSU.name[-3:], name=f"u{ut}"
                        )
                        nc.tensor.matmul(
                            psu, ot[:, us], wo_sb[:, :], start=True, stop=True
                        )
                        t_out = W.tile([128, DIM], F32, tag="uout")
                        nc.vector.tensor_copy(out=t_out, in_=psu)
                        nc.sync.dma_start(out=out_e[us, :], in_=t_out)
                        if stk is not None:
                            stk.__exit__(None, None, None)

                # ---- phase 1: q0 proj, then k/v proj interleaved with
                # qt=0's attention (chunk sc feeds k-chunks 4sc..4sc+3).
                # DMAs are emitted in dependency order so q0/kv0 data
                # arrives first; kv-proj PSUM temps alternate the two
                # spare banks (PSP/PSU) to pipeline matmul vs. evac. ----
                load_tables(0)
                load_w("wq")
                load_w("wqp")
                xt_c = load_xt(0, bass.ts(0, 512))
                load_w("wk")
                load_w("wkp")
                load_w("wv")
                rope_proj(xt_c, bass.ts(0, 512), "q", pools=(PSP, PSU))
                qs0 = bass.ts(0, 512)
                pso = [
                    PSO.tile([HD + 1, 512], F32, tag=f"o{h}", name=f"o0_{h}")
                    for h in range(HPC)
                ]
                for sc in range(NSC):
                    qs = bass.ts(sc, 512)
                    if sc >= 1:
                        load_tables(sc)
                    xt_c = load_xt(8 + sc, qs)
                    # zero-pad rows of this chunk only (a full-tensor
                    # memset would hold up the first scores for ~16us)
                    nc.vector.memset(krA[HD:DPC, qs], 0.0)
                    nc.vector.memset(krB[0:HD, qs], 0.0)
                    ones_sc = vb_sb[:, bass.ds(sc * 4, 4), :].rearrange(
                        "p s (j w) -> p s j w", w=HD + 1
                    )[:, :, :, HD : HD + 1]
                    nc.vector.memset(
                        ones_sc if use_bf16 else ones_sc.bitcast(F32), 1.0
                    )
                    rope_proj(xt_c, qs, "k", pools=(PSP, PSU))
                    pts = []
                    for kcl in range(4):
                        pts.append(scores_exp(sc * 4 + kcl, qs0, 0))
                        v_proj_part(xt_c, sc, kcl, (PSP, PSU)[kcl % 2])
                        if kcl >= 1:
                            attn_pv(pso, sc * 4 + kcl - 1, pts[kcl - 1])
                    if sc < NSC - 1:
                        attn_pv(pso, sc * 4 + 3, pts[3])
                    if sc == 6:
                        # project q1 so qt=1 can start right after qt=0
                        # (normal priority; a mid-loop high_priority insert
                        # inverts proj pool rotation order and deadlocks)
                        xt_q = load_xt(16, bass.ts(1, 512))
                        rope_proj(xt_q, bass.ts(1, 512), "q", pools=(PSP, PSU))
                nc.sync.dma_start(
                    out=wo_sb, in_=wo_e[:] if use_bf16 else wo_e[:].bitcast(F32R)
                )
                pend = (pso, {NKC - 1: pts[3]}, qs0, 0)

                # ---- phase 2: remaining q-chunks, ACT-paced; next q-proj
                # and previous normalize+out-proj run as PE/DVE filler,
                # offset to interleave mid-stream instead of piling up at
                # the qt boundary ----
                for qt in range(1, NSC):
                    qs = bass.ts(qt, 512)
                    if qt + 1 < NSC:
                        with tc.high_priority(offset=-100):
                            xt_q = load_xt(16 + qt, bass.ts(qt + 1, 512))
                            rope_proj(xt_q, bass.ts(qt + 1, 512), "q")
                    pso = [
                        PSO.tile([HD + 1, 512], F32, tag=f"o{h}", name=f"o{qt}_{h}")
                        for h in range(HPC)
                    ]
                    pts = {}
                    for kc in range(NKC):
                        off = "dve" if (kc % 6 == 3 and kc < 28) else False
                        pts[kc] = scores_exp(kc, qs, qt, dve_exp=off)
                        if kc == 2 and pend is not None:
                            # flush the previous q-chunk's trailing PVs,
                            # evacuation and normalize/out-proj AFTER this
                            # chunk's first scores so ScalarE never waits
                            # at the boundary
                            p_pso, p_pts, p_qs, p_qt = pend
                            for k2 in sorted(p_pts):
                                attn_pv(p_pso, k2, p_pts[k2])
                            evac(p_pso, p_qs, p_qt)
                            with tc.high_priority(offset=-110):
                                normalize_project(p_qs, p_qt)
                            pend = None
                        if kc >= 3:
                            attn_pv(pso, kc - 3, pts.pop(kc - 3))
                    pend = (pso, pts, qs, qt)
                p_pso, p_pts, p_qs, p_qt = pend
                for k2 in sorted(p_pts):
                    attn_pv(p_pso, k2, p_pts[k2])
                evac(p_pso, p_qs, p_qt)
                normalize_project(p_qs, p_qt)

    return nc


def _rope_tables():
    freqs = 10000.0 ** (-np.linspace(0.0, 1.0, HALF, endpoint=False))
    theta = np.arange(S, dtype=np.float64)[None, :] * freqs[:, None]  # [32, S]
    cos32 = np.cos(theta)
    sin32 = np.sin(theta)
    cosf = np.tile(np.concatenate([cos32, cos32], axis=0), (HPC, 1))
    sinf = np.tile(np.concatenate([-sin32, sin32], axis=0), (HPC, 1))
    return cosf.astype(np.float32), sinf.astype(np.float32)


def kernel(x, wq_k, wq_b, wk_k, wk_b, wv_k, wv_b, wo_k, wo_b):
    from concourse.bass_utils import run_bass_kernel_spmd

    x = np.asarray(x, np.float32)
    wq_k = np.asarray(wq_k, np.float32)
    wq_b = np.asarray(wq_b, np.float32)
    wk_k = np.asarray(wk_k, np.float32)
    wk_b = np.asarray(wk_b, np.float32)
    wv_k = np.asarray(wv_k, np.float32)
    wv_b = np.asarray(wv_b, np.float32)
    wo_k = np.asarray(wo_k, np.float32)
    wo_b = np.asarray(wo_b, np.float32)

    qk_bias = bool(np.any(wq_b) or np.any(wk_b))
    v_bias = bool(np.any(wv_b))
    use_bf16 = os.environ.get("ATTN_MM_DTYPE", "bf16") != "f32r"

    key = (qk_bias, v_bias, use_bf16)
    if key not in _CACHE:
        nc = _build(qk_bias, v_bias, use_bf16)
        _split_multiwait_drains(nc)
        _CACHE[key] = nc
    nc = _CACHE[key]
    import ml_dtypes

    mmdt = ml_dtypes.bfloat16 if use_bf16 else np.float32

    cosf, sinf = _rope_tables()
    perm = np.r_[HALF:HD, 0:HALF]

    in_maps = []
    for c in range(NCORES):
        b = c // 4
        h0 = HPC * (c % 4)
        hsl = slice(h0, h0 + HPC)
        m = {
            "xt": np.ascontiguousarray(x[b].T).astype(mmdt),
            "wq": np.ascontiguousarray(wq_k[:, hsl, :].reshape(DIM, DPC)).astype(mmdt),
            "wqp": np.ascontiguousarray(wq_k[:, hsl, perm].reshape(DIM, DPC)).astype(mmdt),
            "wk": np.ascontiguousarray(wk_k[:, hsl, :].reshape(DIM, DPC)).astype(mmdt),
            "wkp": np.ascontiguousarray(wk_k[:, hsl, perm].reshape(DIM, DPC)).astype(mmdt),
            "wv": np.ascontiguousarray(wv_k[:, hsl, :].reshape(DIM, DPC)).astype(mmdt),
            "wo": np.ascontiguousarray(wo_k[hsl].reshape(DPC, DIM)).astype(mmdt),
            "cosf": cosf,
            "sinf": sinf,
        }
        if qk_bias:
            m["qb"] = np.ascontiguousarray(wq_b[hsl].reshape(DPC, 1))
            m["qbp"] = np.ascontiguousarray(wq_b[hsl][:, perm].reshape(DPC, 1))
            m["kb"] = np.ascontiguousarray(wk_b[hsl].reshape(DPC, 1))
            m["kbp"] = np.ascontiguousarray(wk_b[hsl][:, perm].reshape(DPC, 1))
        if v_bias:
            m["vb"] = np.ascontiguousarray(wv_b[hsl].reshape(1, DPC))
        in_maps.append(m)

    res = run_bass_kernel_spmd(nc, in_maps, list(range(NCORES)))

    out = np.zeros((B, S, DIM), np.float32)
    for c in range(NCORES):
        out[c // 4] += res.results[c]["out"]
    out += wo_b[None, None, :]
    return out


# revision 35
# speedup vs baseline: 1.1976x; 1.0186x over previous
"""Trainium2 Bass kernel for nn_Attention_90074054132266.

Full multi-head attention (B=2, S=4096, D=512, H=8, HD=64) with RoPE on
q/k, sharded over 8 NeuronCores: batch x head-pair (data parallel over
batch, tensor parallel over heads; core c handles batch c//4, heads
2*(c%4), 2*(c%4)+1). Each core computes a partial output projection
(its 2 heads' contribution); the host sums the 4 per-batch partials
(the "all-reduce") and adds wo_b.

Per-core device algorithm (everything stored transposed, f32/bf16):
  - host passes x[b].T, so projections q^T/k^T = wq^T-chunks @ x^T run
    as N=512 matmuls.
  - RoPE via duplicated projections with half-swapped weight columns
    (q2^T[d] = q^T[(d+32)%64 per head]) + sign-baked cos/sin tables.
  - scores per 128-k-chunk for BOTH heads in one PSUM tile
    S^T[k, (h, q)] = [128, 1024]; exp(S/8) on ScalarE out of PSUM
    (scale folded into the activation; no max subtraction: scores are
    ~N(0,1), exp is safe in fp32).  5 of 32 k-chunks per q-chunk use a
    VectorE int16-Schraudolph fast exp (bitcast(int16(A*x+B)) in bf16,
    ~3%% max element err) to offload the saturated ScalarE.
  - P@V accumulates per-head O'^T[65, 512] over 32 k-chunks; V' has a
    ones column per head: row 64 = softmax denominator Z for free.
    PVs trail their exp by 3 k-chunks so a slow consumer never stalls
    the PE's static instruction order.
  - Z path (off the PSUM-release path): evac O'/Z with plain copies,
    then Z -> DRAM -> partition-broadcast izb[128, 512] -> reciprocal,
    normalize ot in place, fused single-matmul out-projection per
    128-row tile (U = ot.T @ wo contracts both heads at once).  All of
    it runs as low-priority filler during the next q-chunk.
  - schedule: k/v projection is interleaved with qt=0's attention
    (chunk sc feeds k-chunks 4sc..4sc+3); each later q-chunk flushes
    the previous chunk's trailing PVs/evac/normalize AFTER its first
    scores so ScalarE never idles at chunk boundaries; next q-chunk's
    q-projection runs as PE filler at a small priority offset so it
    interleaves mid-stream.
  - PSUM budget (8 banks): scores 2x2 + O' 2x1 + projection 1 +
    out-projection 1.  Filler pools are separate so buffer-rotation
    WARs cannot invert the priority order (that deadlocks the queues).
"""

import os
import sys

sys.path.insert(0, "/opt/trn_rl_repo")

import numpy as np

B, S, DIM, HEADS, HD = 2, 4096, 512, 8, 64
HALF = HD // 2
NCORES = 8
HPC = 2  # heads per core
DPC = HPC * HD  # 128 projection columns per core
NSC = S // 512  # 8 q-column chunks of 512
NKC = S // 128  # 32 k-chunks of 128
NUT = S // 128  # 32 q-row tiles of 128

_CACHE = {}


def _split_multiwait_drains(nc):
    """The walrus build in this container rejects any instruction with
    more than one sync-wait ("Too many sync wait commands"). Hoist the
    extra waits onto preceding same-engine NoOps, leaving one wait on
    the original instruction."""
    import bass_rust
    import concourse.mybir as mybir

    for fn in nc.m.functions:
        for bb in fn.blocks:
            new_insts = []
            changed = False
            for inst in bb.instructions:
                si = getattr(inst, "sync_info", None)
                if si is not None and len(si.on_wait) > 1:
                    waits = list(si.on_wait)
                    for k, w in enumerate(waits[:-1]):
                        d = mybir.InstNoOp(name=f"{inst.name}w{k}", ins=[], outs=[])
                        d.engine = inst.engine
                        d.sync_info = bass_rust.SyncInfo(on_wait=[w], on_update=[])
                        new_insts.append(d)
                    inst.sync_info = bass_rust.SyncInfo(
                        on_wait=[waits[-1]], on_update=list(si.on_update)
                    )
                    changed = True
                new_insts.append(inst)
            if changed:
                bb.instructions = new_insts


def _build(qk_bias, v_bias, use_bf16=True):
    import concourse.bass as bass
    import concourse.tile as tile
    from concourse import mybir

    F32 = mybir.dt.float32
    F32R = mybir.dt.float32r
    BF16 = mybir.dt.bfloat16
    MMD = BF16 if use_bf16 else F32R   # matmul operand dtype (SBUF tiles)
    MME = BF16 if use_bf16 else F32    # DRAM dtype for matmul inputs
    EXP = mybir.ActivationFunctionType.Exp
    MUL = mybir.AluOpType.mult
    ADD = mybir.AluOpType.add
    I16 = mybir.dt.int16
    # Schraudolph fast-exp in bf16: bitcast(int16(A*x + B)) ~= exp(x/8)
    # (max rel err ~3%; used on a few tiles per q-chunk to offload the
    # saturated ScalarE onto the idle VectorE)
    SCH_A = float(2.0**7 / np.log(2.0) * 0.125)
    SCH_B = float(127.0 * 2.0**7 - 366393.0 / 65536.0)

    nc = bass.Bass("TRN2")

    xt_e = nc.declare_dram_parameter("xt", [DIM, S], MME, isOutput=False)
    w_e = {}
    for name in ("wq", "wqp", "wk", "wkp", "wv"):
        w_e[name] = nc.declare_dram_parameter(name, [128, 4, DPC], MME, isOutput=False)
    wo_e = nc.declare_dram_parameter("wo", [DPC, DIM], MME, isOutput=False)
    cos_e = nc.declare_dram_parameter("cosf", [DPC, S], F32, isOutput=False)
    sin_e = nc.declare_dram_parameter("sinf", [DPC, S], F32, isOutput=False)
    b_e = {}
    if qk_bias:
        for name in ("qb", "qbp", "kb", "kbp"):
            b_e[name] = nc.declare_dram_parameter(name, [DPC, 1], F32, isOutput=False)
    if v_bias:
        b_e["vb"] = nc.declare_dram_parameter("vb", [1, DPC], F32, isOutput=False)
    out_e = nc.declare_dram_parameter("out", [S, DIM], F32, isOutput=True)

    with tile.TileContext(nc) as tc:
        with (
            tc.tile_pool(name="persist", bufs=1) as P,
            tc.tile_pool(name="work", bufs=2) as W,
            tc.tile_pool(name="ptp", bufs=8) as PT,
        ):
            # ---- persistent SBUF tensors ----
            qr = P.tile([DPC, S], MMD, tag="qr")  # rotated q^T
            # rotated k^T, zero-padded per head to full K=128 contraction
            # (row-masked K=64 matmuls don't count as PE-busy for the HAM
            # clock gate; mixing them with PV pins the PE at 1.2 GHz)
            krA = P.tile([DPC, S], MMD, tag="krA")
            krB = P.tile([DPC, S], MMD, tag="krB")
            # Z rows staging (head h at partition 32h; DVE outputs must be
            # quadrant-aligned)
            zrow = P.tile([33, S], F32, tag="zrow")
            zrowB = P.tile([1, S], F32, tag="zrowB")  # tail: h1 Z at partition 0
            # V' rows: per k-chunk st, V[k, :] for head A cols 0:64 + ones
            # col 64, head B cols 65:129 + ones col 129.
            VW = 2 * (HD + 1)  # 130
            vb_sb = P.tile([128, NKC, VW], MMD, tag="vboth")
            # normalized O^T packed: rows 0:64 head A, 64:128 head B
            ot = P.tile([DPC, S], MMD, tag="ot")
            wo_sb = P.tile([DPC, DIM], MMD, tag="wo")
            # masked ones rows for the tail's PE Z-broadcast (K=1 matmuls)
            onesA = P.tile([1, 128], F32, tag="onesA")
            onesB = P.tile([1, 128], F32, tag="onesB")
            nc.vector.memset(onesA[0:1, 0:HD], 1.0)
            nc.vector.memset(onesA[0:1, HD:DPC], 0.0)
            nc.vector.memset(onesB[0:1, 0:HD], 0.0)
            nc.vector.memset(onesB[0:1, HD:DPC], 1.0)


            bias_sb = {}
            if qk_bias:
                for name in ("qb", "qbp", "kb", "kbp"):
                    t = P.tile([DPC, 1], F32, tag=name)
                    nc.sync.dma_start(out=t, in_=b_e[name][:])
                    bias_sb[name] = t
            if v_bias:
                vbias_bc = P.tile([128, DPC], F32, tag="vbias")
                src = bass.AP(
                    tensor=b_e["vb"].tensor,
                    offset=b_e["vb"].offset,
                    ap=[[0, 128], [1, DPC]],
                )
                nc.sync.dma_start(out=vbias_bc, in_=src)

            # ---- PSUM budget (8 banks): scores 2x2 + pso 2x1 + proj 1
            # + out-proj U 1 (separate pools so filler rotations decouple)
            with (
                tc.tile_pool(name="xtp", bufs=2) as XT,
                tc.tile_pool(name="wpool", bufs=1) as WP,
                tc.tile_pool(name="psp", bufs=1, space="PSUM") as PSP,
                tc.tile_pool(name="psu", bufs=1, space="PSUM") as PSU,
                tc.tile_pool(name="pss", bufs=2, space="PSUM") as PSS,
                tc.tile_pool(name="pso", bufs=1, space="PSUM") as PSO,
            ):
                cos_sb = WP.tile([DPC, S], F32, tag="cos")
                sin_sb = WP.tile([DPC, S], F32, tag="sin")

                def load_tables(sc):
                    qs_ = bass.ts(sc, 512)
                    nc.sync.dma_start(out=cos_sb[:, qs_], in_=cos_e[:, qs_])
                    nc.sync.dma_start(out=sin_sb[:, qs_], in_=sin_e[:, qs_])

                w_sb = {}

                def load_w(name):
                    t = WP.tile([128, 4, DPC], MMD, tag=name)
                    nc.sync.dma_start(
                        out=t,
                        in_=w_e[name][:] if use_bf16 else w_e[name][:].bitcast(F32R),
                    )
                    w_sb[name] = t

                xt_r = xt_e[:].rearrange("(c p) s -> c p s", p=128)

                def load_xt(sc, qs):
                    xt_c = []
                    for c in range(4):
                        t = XT.tile(
                            [128, 512], MMD, tag=f"xt{c}", name=f"xt{c}_{sc}"
                        )
                        nc.sync.dma_start(
                            out=t,
                            in_=xt_r[c, :, qs]
                            if use_bf16
                            else xt_r[c, :, qs].bitcast(F32R),
                        )
                        xt_c.append(t)
                    return xt_c

                def rope_proj(xt_c, qs, which, pools=None):
                    pools = pools or (PSP, PSP)
                    # one 512-col chunk of rotated q^T or (split) k^T
                    wn, wpn, bn, bpn = (
                        ("wq", "wqp", "qb", "qbp")
                        if which == "q"
                        else ("wk", "wkp", "kb", "kbp")
                    )
                    ps1 = pools[0].tile(
                        [128, 512], F32, tag=pools[0].name[-3:], name=f"p1_{which}{qs}"
                    )
                    for c in range(4):
                        nc.tensor.matmul(
                            ps1,
                            w_sb[wn][:, c, :],
                            xt_c[c][:],
                            start=(c == 0),
                            stop=(c == 3),
                        )
                    if qk_bias:
                        s1 = W.tile([128, 512], F32, tag="rope1")
                        nc.vector.tensor_scalar_add(s1, ps1, bias_sb[bn])
                    else:
                        s1 = ps1
                    t3 = W.tile([128, 512], F32, tag="rope3")
                    nc.vector.tensor_tensor(out=t3, in0=s1, in1=cos_sb[:, qs], op=MUL)
                    ps2 = pools[1].tile(
                        [128, 512], F32, tag=pools[1].name[-3:], name=f"p2_{which}{qs}"
                    )
                    for c in range(4):
                        nc.tensor.matmul(
                            ps2,
                            w_sb[wpn][:, c, :],
                            xt_c[c][:],
                            start=(c == 0),
                            stop=(c == 3),
                        )
                    if qk_bias:
                        s2 = W.tile([128, 512], F32, tag="rope2")
                        nc.vector.tensor_scalar_add(s2, ps2, bias_sb[bpn])
                    else:
                        s2 = ps2
                    t4 = W.tile([128, 512], F32, tag="rope4")
                    nc.vector.tensor_tensor(out=t4, in0=s2, in1=sin_sb[:, qs], op=MUL)
                    if which == "q":
                        nc.vector.tensor_tensor(out=qr[:, qs], in0=t3, in1=t4, op=ADD)
                    else:
                        nc.vector.tensor_tensor(
                            out=krA[0:HD, qs], in0=t3[0:HD, :], in1=t4[0:HD, :], op=ADD
                        )
                        nc.vector.tensor_tensor(
                            out=krB[HD:DPC, qs],
                            in0=t3[HD:DPC, :],
                            in1=t4[HD:DPC, :],
                            op=ADD,
                        )

                def v_proj_part(xt_c, sc, stl, pl):
                    st = sc * 4 + stl
                    psv = pl.tile([128, 512], F32, tag=pl.name[-3:], name=f"pv{st}")
                    for c in range(4):
                        nc.tensor.matmul(
                            psv[:, 0:128],
                            xt_c[c][:, bass.ts(stl, 128)],
                            w_sb["wv"][:, c, :],
                            start=(c == 0),
                            stop=(c == 3),
                        )
                    dsts = vb_sb[:, st, :].rearrange(
                        "p (j w) -> p j w", w=HD + 1
                    )[:, :, 0:HD]
                    if v_bias:
                        nc.vector.tensor_tensor(
                            out=dsts, in0=psv[:, 0:128], in1=vbias_bc, op=ADD
                        )
                    else:
                        nc.vector.tensor_copy(out=dsts, in_=psv[:, 0:128])

                # DRAM scratch for the 1/Z partition-broadcast bounce
                zscr = nc.dram_tensor("zscr", [HPC, S], F32)
                zscr_ap = zscr[:]

                def scores_exp(kc, qs, qt, dve_exp=False):
                    # scores for both heads of k-chunk kc, then exp
                    pss_t = PSS.tile([128, 1024], F32, tag="s", name=f"ss{qt}_{kc}")
                    nc.tensor.matmul(
                        pss_t[:, 0:512],
                        krA[:, bass.ts(kc, 128)],
                        qr[:, qs],
                        start=True,
                        stop=True,
                    )
                    nc.tensor.matmul(
                        pss_t[:, 512:1024],
                        krB[:, bass.ts(kc, 128)],
                        qr[:, qs],
                        start=True,
                        stop=True,
                    )
                    if dve_exp:
                        pti = PT.tile(
                            [128, 1024], I16, tag="pti", name=f"pti{qt}_{kc}"
                        )
                        eng = nc.gpsimd if dve_exp == "gps" else nc.vector
                        eng.tensor_scalar(
                            out=pti,
                            in0=pss_t,
                            scalar1=SCH_A,
                            scalar2=SCH_B,
                            op0=MUL,
                            op1=ADD,
                        )
                        return pti[:].bitcast(BF16)
                    pt = PT.tile([128, 1024], MMD, tag="pt", name=f"pt{qt}_{kc}")
                    nc.scalar.activation(out=pt, in_=pss_t, func=EXP, scale=0.125)
                    return pt

                def attn_pv(pso, kc, pt):
                    for h in range(HPC):
                        nc.tensor.matmul(
                            pso[h],
                            vb_sb[:, kc, bass.ds(h * (HD + 1), HD + 1)],
                            pt[:, bass.ts(h, 512)],
                            start=(kc == 0),
                            stop=(kc == NKC - 1),
                        )

                def attn_kc(pso, kc, qs, qt):
                    attn_pv(pso, kc, scores_exp(kc, qs, qt))

                def evac(pso, qs, qt, last=False):
                    # fast PSUM evacuation: unnormalized O' into packed ot,
                    # Z rows into zrow; releases the pso banks quickly so
                    # the next q-chunk's PV never stalls.  On the final
                    # chunk the h0 copy runs on the idle ScalarE so the
                    # two O' copies overlap.
                    for h in range(HPC):
                        if last and h == 0:
                            nc.scalar.copy(
                                out=ot[0:HD, qs], in_=pso[0][0:HD, :]
                            )
                        else:
                            nc.vector.tensor_copy(
                                out=ot[h * HD : (h + 1) * HD, qs],
                                in_=pso[h][0:HD, :],
                            )
                        zdst = (
                            zrowB[0:1, qs]
                            if (last and h == 1)
                            else zrow[32 * h : 32 * h + 1, qs]
                        )
                        nc.vector.tensor_copy(
                            out=zdst, in_=pso[h][HD : HD + 1, :]
                        )

                def normalize_project(qs, qt, stagger=False, last=False):
                    # Z -> DRAM -> partition-broadcast -> 1/Z, normalize ot
                    # in place, then the fused single-matmul out-projection.
                    # Latency-tolerant: runs as filler during the next qt.
                    if last:
                        # PE broadcast of the Z rows into a spare PSUM bank
                        # via masked K=1 ones-matmuls: skips the ~5us DRAM
                        # bounce.  Safe only here: the tail already sits in
                        # the HAM half-clock window, so the row-masked
                        # matmuls cannot newly de-boost the PE.
                        izbz = PSP.tile([128, 512], F32, tag="psp", name="izbzl")
                        nc.tensor.matmul(
                            izbz,
                            onesA[:],
                            zrow[0:1, qs],
                            start=True,
                            stop=False,
                        )
                        nc.tensor.matmul(
                            izbz,
                            onesB[:],
                            zrowB[0:1, qs],
                            start=False,
                            stop=True,
                        )
                    else:
                        for h in range(HPC):
                            nc.sync.dma_start(
                                out=zscr[h : h + 1, qs],
                                in_=zrow[32 * h : 32 * h + 1, qs],
                            )
                        izbz = W.tile(
                            [128, 512], F32, tag="izbz", name=f"izbz{qt}"
                        )
                        for h in range(HPC):
                            zsl = zscr_ap[h : h + 1, qs]
                            src = bass.AP(
                                tensor=zsl.tensor,
                                offset=zsl.offset,
                                ap=[[0, HD], [1, 512]],
                            )
                            nc.sync.dma_start(
                                out=izbz[h * HD : (h + 1) * HD, :], in_=src
                            )
                    izb = W.tile([128, 512], F32, tag="izb", name=f"izb{qt}")
                    for utl in range(4):
                        stk = (
                            tc.high_priority(offset=-(15 + 36 * utl))
                            if stagger
                            else None
                        )
                        if stk is not None:
                            stk.__enter__()
                        ut = qt * 4 + utl
                        us = bass.ts(ut, 128)
                        cs = bass.ts(utl, 128)
                        nc.vector.reciprocal(out=izb[:, cs], in_=izbz[:, cs])
                        for h in range(HPC):
                            otb = ot[h * HD : (h + 1) * HD, qs].rearrange(
                                "p (u c) -> p u c", c=128
                            )[:, utl, :]
                            nc.vector.tensor_tensor(
                                out=otb,
                                in0=otb,
                                in1=izb[h * HD : (h + 1) * HD, cs],
                                op=MUL,
                            )
                        psu = PSU.tile(
                            [128, DIM], F32, tag=PSU.name[-3:], name=f"u{ut}"
                        )
                        nc.tensor.matmul(
                            psu, ot[:, us], wo_sb[:, :], start=True, stop=True
                        )
                        t_out = W.tile([128, DIM], F32, tag="uout")
                        if last:
                            nc.scalar.copy(out=t_out, in_=psu)
                        else:
                            nc.vector.tensor_copy(out=t_out, in_=psu)
                        nc.sync.dma_start(out=out_e[us, :], in_=t_out)
                        if stk is not None:
                            stk.__exit__(None, None, None)

                # ---- phase 1: q0 proj, then k/v proj interleaved with
                # qt=0's attention (chunk sc feeds k-chunks 4sc..4sc+3).
                # DMAs are emitted in dependency order so q0/kv0 data
                # arrives first; kv-proj PSUM temps alternate the two
                # spare banks (PSP/PSU) to pipeline matmul vs. evac. ----
                load_tables(0)
                load_w("wq")
                load_w("wqp")
                xt_c = load_xt(0, bass.ts(0, 512))
                load_w("wk")
                load_w("wkp")
                load_w("wv")
                rope_proj(xt_c, bass.ts(0, 512), "q", pools=(PSP, PSU))
                qs0 = bass.ts(0, 512)
                pso = [
                    PSO.tile([HD + 1, 512], F32, tag=f"o{h}", name=f"o0_{h}")
                    for h in range(HPC)
                ]
                for sc in range(NSC):
                    qs = bass.ts(sc, 512)
                    if sc >= 1:
                        load_tables(sc)
                    xt_c = load_xt(8 + sc, qs)
                    # zero-pad rows of this chunk only (a full-tensor
                    # memset would hold up the first scores for ~16us)
                    nc.vector.memset(krA[HD:DPC, qs], 0.0)
                    nc.vector.memset(krB[0:HD, qs], 0.0)
                    ones_sc = vb_sb[:, bass.ds(sc * 4, 4), :].rearrange(
                        "p s (j w) -> p s j w", w=HD + 1
                    )[:, :, :, HD : HD + 1]
                    nc.vector.memset(
                        ones_sc if use_bf16 else ones_sc.bitcast(F32), 1.0
                    )
                    rope_proj(xt_c, qs, "k", pools=(PSP, PSU))
                    pts = []
                    for kcl in range(4):
                        pts.append(scores_exp(sc * 4 + kcl, qs0, 0))
                        v_proj_part(xt_c, sc, kcl, (PSP, PSU)[kcl % 2])
                        if kcl >= 1:
                            attn_pv(pso, sc * 4 + kcl - 1, pts[kcl - 1])
                    if sc < NSC - 1:
                        attn_pv(pso, sc * 4 + 3, pts[3])
                    if sc == 6:
                        # project q1 so qt=1 can start right after qt=0
                        # (normal priority; a mid-loop high_priority insert
                        # inverts proj pool rotation order and deadlocks)
                        xt_q = load_xt(16, bass.ts(1, 512))
                        rope_proj(xt_q, bass.ts(1, 512), "q", pools=(PSP, PSU))
                nc.sync.dma_start(
                    out=wo_sb, in_=wo_e[:] if use_bf16 else wo_e[:].bitcast(F32R)
                )
                pend = (pso, {NKC - 1: pts[3]}, qs0, 0)

                # ---- phase 2: remaining q-chunks, ACT-paced; next q-proj
                # and previous normalize+out-proj run as PE/DVE filler,
                # offset to interleave mid-stream instead of piling up at
                # the qt boundary ----
                for qt in range(1, NSC):
                    qs = bass.ts(qt, 512)
                    if qt + 1 < NSC:
                        with tc.high_priority(offset=-100):
                            xt_q = load_xt(16 + qt, bass.ts(qt + 1, 512))
                            rope_proj(xt_q, bass.ts(qt + 1, 512), "q")
                    pso = [
                        PSO.tile([HD + 1, 512], F32, tag=f"o{h}", name=f"o{qt}_{h}")
                        for h in range(HPC)
                    ]
                    pts = {}
                    for kc in range(NKC):
                        off = "dve" if (kc % 6 == 3 and kc < 28) else False
                        pts[kc] = scores_exp(kc, qs, qt, dve_exp=off)
                        if kc == 2 and pend is not None:
                            # flush the previous q-chunk's trailing PVs,
                            # evacuation and normalize/out-proj AFTER this
                            # chunk's first scores so ScalarE never waits
                            # at the boundary
                            p_pso, p_pts, p_qs, p_qt = pend
                            for k2 in sorted(p_pts):
                                attn_pv(p_pso, k2, p_pts[k2])
                            evac(p_pso, p_qs, p_qt)
                            with tc.high_priority(offset=-110):
                                normalize_project(p_qs, p_qt)
                            pend = None
                        if kc >= 3:
                            attn_pv(pso, kc - 3, pts.pop(kc - 3))
                    pend = (pso, pts, qs, qt)
                p_pso, p_pts, p_qs, p_qt = pend
                for k2 in sorted(p_pts):
                    attn_pv(p_pso, k2, p_pts[k2])
                evac(p_pso, p_qs, p_qt, last=True)
                normalize_project(p_qs, p_qt, last=True)

    return nc


def _rope_tables():
    freqs = 10000.0 ** (-np.linspace(0.0, 1.0, HALF, endpoint=False))
    theta = np.arange(S, dtype=np.float64)[None, :] * freqs[:, None]  # [32, S]
    cos32 = np.cos(theta)
    sin32 = np.sin(theta)
    cosf = np.tile(np.concatenate([cos32, cos32], axis=0), (HPC, 1))
    sinf = np.tile(np.concatenate([-sin32, sin32], axis=0), (HPC, 1))
    return cosf.astype(np.float32), sinf.astype(np.float32)


def _wlay(w, mmdt):
    # device weight layout [p, c, m]: row c*128+p of the [DIM, DPC] matrix
    w = np.ascontiguousarray(w.reshape(DIM, DPC))
    return np.ascontiguousarray(
        w.reshape(4, 128, DPC).transpose(1, 0, 2)
    ).astype(mmdt)


def kernel(x, wq_k, wq_b, wk_k, wk_b, wv_k, wv_b, wo_k, wo_b):
    from concourse.bass_utils import run_bass_kernel_spmd

    x = np.asarray(x, np.float32)
    wq_k = np.asarray(wq_k, np.float32)
    wq_b = np.asarray(wq_b, np.float32)
    wk_k = np.asarray(wk_k, np.float32)
    wk_b = np.asarray(wk_b, np.float32)
    wv_k = np.asarray(wv_k, np.float32)
    wv_b = np.asarray(wv_b, np.float32)
    wo_k = np.asarray(wo_k, np.float32)
    wo_b = np.asarray(wo_b, np.float32)

    qk_bias = bool(np.any(wq_b) or np.any(wk_b))
    v_bias = bool(np.any(wv_b))
    use_bf16 = os.environ.get("ATTN_MM_DTYPE", "bf16") != "f32r"

    key = (qk_bias, v_bias, use_bf16)
    if key not in _CACHE:
        nc = _build(qk_bias, v_bias, use_bf16)
        _split_multiwait_drains(nc)
        _CACHE[key] = nc
    nc = _CACHE[key]
    import ml_dtypes

    mmdt = ml_dtypes.bfloat16 if use_bf16 else np.float32

    cosf, sinf = _rope_tables()
    perm = np.r_[HALF:HD, 0:HALF]

    in_maps = []
    for c in range(NCORES):
        b = c // 4
        h0 = HPC * (c % 4)
        hsl = slice(h0, h0 + HPC)
        m = {
            "xt": np.ascontiguousarray(x[b].T).astype(mmdt),
            "wq": _wlay(wq_k[:, hsl, :], mmdt),
            "wqp": _wlay(wq_k[:, hsl, perm], mmdt),
            "wk": _wlay(wk_k[:, hsl, :], mmdt),
            "wkp": _wlay(wk_k[:, hsl, perm], mmdt),
            "wv": _wlay(wv_k[:, hsl, :], mmdt),
            "wo": np.ascontiguousarray(wo_k[hsl].reshape(DPC, DIM)).astype(mmdt),
            "cosf": cosf,
            "sinf": sinf,
        }
        if qk_bias:
            m["qb"] = np.ascontiguousarray(wq_b[hsl].reshape(DPC, 1))
            m["qbp"] = np.ascontiguousarray(wq_b[hsl][:, perm].reshape(DPC, 1))
            m["kb"] = np.ascontiguousarray(wk_b[hsl].reshape(DPC, 1))
            m["kbp"] = np.ascontiguousarray(wk_b[hsl][:, perm].reshape(DPC, 1))
        if v_bias:
            m["vb"] = np.ascontiguousarray(wv_b[hsl].reshape(1, DPC))
        in_maps.append(m)

    res = run_bass_kernel_spmd(nc, in_maps, list(range(NCORES)))

    out = np.zeros((B, S, DIM), np.float32)
    for c in range(NCORES):
        out[c // 4] += res.results[c]["out"]
    out += wo_b[None, None, :]
    return out
